# revision 4
# baseline (speedup 1.0000x reference)
"""DimeNet++ interaction block on 8 Trainium2 NeuronCores (Bass/Tile).

Strategy (matches the edge/triplet data-parallel sharding hint):
  * Edges are split contiguously 8 ways (50K edges/core).
  * Launch 1 (edge-parallel): each core computes its shard of the gather
    table  x_kj_down = silu((silu(x@W_kj+b) * ((rbf@W_rbf1)@W_rbf2)) @ W_down)
    AND the pre-activated  x_ji = silu(x@W_ji + b_ji)  in transposed layout.
    The host concatenates the 8 table shards into the full [E, I] table and
    permutes x_ji into the per-core group order used by launch 2.
  * Triplets are routed to the core that owns their idx_ji edge (host sorts
    triplets by idx_ji).  Within a core, edges are bucketed by degree class
    D and packed into 128-edge groups; each edge's triplet list is padded to
    D slots (padded-CSR).  Per group the device does:
      - one indirect DMA gather of 128*D rows from the replicated table
      - ntrip small matmuls  sbf_i = sbfT_chunk.T @ (W_sbf1@W_sbf2), two
        triples packed per PSUM bank
      - DVE multiply  m = gathered * sbf_i  (one op per packed pair)
      - DVE tensor_reduce over the D axis -> agg [128 edges, I]
      - PE transpose of agg -> [I, 128] for the downstream matmuls
  * The tail MLP (W_up, residual stack) runs per 1024-edge macro-tile
    entirely in transposed-activation layout; the host undoes the edge
    permutation / transposition when assembling the full output.

Everything the device computes is fp16-in/fp32-accumulate; the only host
arithmetic is the (associativity-exact) folding of W_rbf1@W_rbf2 and
W_sbf1@W_sbf2.
"""

import math
import sys
from contextlib import ExitStack

for _p in ("/opt/trn_rl_repo",):
    if _p not in sys.path:
        sys.path.insert(0, _p)

import numpy as np

import concourse.bass as bass
import concourse.mybir as mybir
import concourse.tile as tile
from concourse import bacc
from concourse.bass_utils import run_bass_kernel_spmd
from concourse.masks import make_identity

F32 = mybir.dt.float32
I32 = mybir.dt.int32
SILU = mybir.ActivationFunctionType.Silu
MULT = mybir.AluOpType.mult
ADD = mybir.AluOpType.add
AXIS_X = mybir.AxisListType.X

N_CORES = 8
F16 = mybir.dt.float16
WDT = F16          # matmul-path dtype (single-pass PE, half DMA bytes)
MACRO_G = 8        # groups per tail macro-tile (8 * 128 = 1024 edges)
CD_LAYOUT = True   # store m feature-major so the reduce reads stride-1


def _np_wdt():
    return np.float16 if WDT == F16 else np.float32


def _mm(nc, out, lhsT, rhs, start, stop):
    nc.tensor.matmul(out=out, lhsT=lhsT, rhs=rhs, start=start, stop=stop)


# --------------------------------------------------------------------------
# device program builders
# --------------------------------------------------------------------------

def _dram(nc, name, shape, dtype=F32, out=False):
    kind = "ExternalOutput" if out else "ExternalInput"
    return nc.dram_tensor(name, list(shape), dtype, kind=kind).ap()


def _load_weight_chunks(nc, pool, dram_ap, tag, dtype=None):
    """Load a [K, M] weight into SBUF as 128-partition K-chunks."""
    K = dram_ap.shape[0]
    if dtype is None:
        dtype = WDT
    tiles = []
    for k0 in range(0, K, 128):
        ksz = min(128, K - k0)
        t = pool.tile([ksz, dram_ap.shape[1]], dtype, tag=f"{tag}_{k0}")
        nc.sync.dma_start(out=t[:], in_=dram_ap[k0:k0 + ksz, :])
        tiles.append((t, ksz))
    return tiles


def _load_bias_chunks(nc, pool, dram_ap, tag):
    """Load a [M] bias into SBUF as per-partition [msz, 1] chunks."""
    M = dram_ap.shape[0]
    tiles = []
    for m0 in range(0, M, 128):
        msz = min(128, M - m0)
        t = pool.tile([msz, 1], F32, tag=f"{tag}_{m0}")
        nc.sync.dma_start(out=t[:], in_=dram_ap[m0:m0 + msz, None])
        tiles.append(t)
    return tiles


def build_launch1(Epc, H, NR, I):
    """Per-core: xT [H, Epc], rbfT [NR, Epc] ->
    tbl_out [Epc, I] (row major) and xjiT_out [H, Epc] (transposed)."""
    TILE = 1024
    assert Epc % TILE == 0

    nc = bacc.Bacc("TRN2", target_bir_lowering=False, debug=False)
    xT = _dram(nc, "xT", [H, Epc], WDT)
    rbfT = _dram(nc, "rbfT", [NR, Epc], WDT)
    w_kj = _dram(nc, "W_kj", [H, H], WDT)
    b_kj = _dram(nc, "b_kj", [H])
    w_ji = _dram(nc, "W_ji", [H, H], WDT)
    b_ji = _dram(nc, "b_ji", [H])
    w_rbf = _dram(nc, "W_rbf", [NR, H], WDT)
    w_down = _dram(nc, "W_down", [H, I], WDT)
    tbl_out = _dram(nc, "tbl_out", [Epc, I], WDT, out=True)
    xji_out = _dram(nc, "xji_out", [H, Epc], WDT, out=True)

    with tile.TileContext(nc) as tc, ExitStack() as ctx:
        const = ctx.enter_context(tc.tile_pool(name="const", bufs=1))
        wkj_t = _load_weight_chunks(nc, const, w_kj, "wkj")
        bkj_t = _load_bias_chunks(nc, const, b_kj, "bkj")
        wji_t = _load_weight_chunks(nc, const, w_ji, "wji")
        bji_t = _load_bias_chunks(nc, const, b_ji, "bji")
        wrbf_t = _load_weight_chunks(nc, const, w_rbf, "wrbf")
        wdown_t = _load_weight_chunks(nc, const, w_down, "wdown")

        xp = ctx.enter_context(tc.tile_pool(name="xp", bufs=3))
        work = ctx.enter_context(tc.tile_pool(name="work", bufs=2))
        outp = ctx.enter_context(tc.tile_pool(name="outp", bufs=3))
        # one rotating PSUM tag (2 banks/slot * 3) + down bank * 2  = 8 banks
        ps_big = ctx.enter_context(tc.tile_pool(name="ps_big", bufs=3, space="PSUM"))
        ps_d = ctx.enter_context(tc.tile_pool(name="ps_d", bufs=2, space="PSUM"))

        for t0 in range(0, Epc, TILE):
            xts = []
            for m0 in range(0, H, 128):
                msz = min(128, H - m0)
                xt = xp.tile([msz, TILE], WDT, tag=f"x_{m0}")
                nc.sync.dma_start(out=xt[:], in_=xT[m0:m0 + msz, t0:t0 + TILE])
                xts.append((xt, msz))
            rbt = xp.tile([NR, TILE], WDT, tag="rbf")
            nc.sync.dma_start(out=rbt[:], in_=rbfT[:, t0:t0 + TILE])

            xms = []
            for mi, m0 in enumerate(range(0, H, 128)):
                msz = min(128, H - m0)
                # rbf_h chunk
                ps_r = ps_big.tile([msz, TILE], F32, tag="ps")
                for sub in range(TILE // 512):
                    sl = slice(sub * 512, (sub + 1) * 512)
                    _mm(nc, ps_r[:, sl], wrbf_t[0][0][:, m0:m0 + msz], rbt[:, sl],
                        True, True)
                rh = work.tile([msz, TILE], WDT, tag=f"rh_{m0}")
                nc.vector.tensor_copy(rh[:], ps_r[:])
                # x_kj chunk
                ps_k = ps_big.tile([msz, TILE], F32, tag="ps")
                for sub in range(TILE // 512):
                    sl = slice(sub * 512, (sub + 1) * 512)
                    for ki, (wt, ksz) in enumerate(wkj_t):
                        _mm(nc, ps_k[:, sl], wt[:, m0:m0 + msz], xts[ki][0][:, sl],
                            ki == 0, ki == len(wkj_t) - 1)
                xk = work.tile([msz, TILE], WDT, tag=f"xk_{m0}")
                nc.scalar.activation(out=xk[:], in_=ps_k[:], func=SILU,
                                     bias=bkj_t[mi][:])
                xm = work.tile([msz, TILE], WDT, tag=f"xm_{m0}")
                nc.vector.tensor_tensor(out=xm[:], in0=xk[:], in1=rh[:], op=MULT)
                xms.append((xm, msz))
                # x_ji chunk
                ps_j = ps_big.tile([msz, TILE], F32, tag="ps")
                for sub in range(TILE // 512):
                    sl = slice(sub * 512, (sub + 1) * 512)
                    for ki, (wt, ksz) in enumerate(wji_t):
                        _mm(nc, ps_j[:, sl], wt[:, m0:m0 + msz], xts[ki][0][:, sl],
                            ki == 0, ki == len(wji_t) - 1)
                xj = outp.tile([msz, TILE], WDT, tag=f"xj_{m0}")
                nc.scalar.activation(out=xj[:], in_=ps_j[:], func=SILU,
                                     bias=bji_t[mi][:])
                nc.sync.dma_start(out=xji_out[m0:m0 + msz, t0:t0 + TILE], in_=xj[:])

            # x_kj_down rows: 4 x 128-edge subtiles packed into one PSUM bank
            for half in range(TILE // 512):
                psd = ps_d.tile([128, 256], F32, tag="psd")
                for j in range(4):
                    sl = slice(half * 512 + j * 128, half * 512 + (j + 1) * 128)
                    for ki, (xm, ksz) in enumerate(xms):
                        _mm(nc, psd[:, j * I:(j + 1) * I], xm[:, sl],
                            wdown_t[ki][0][:], ki == 0, ki == len(xms) - 1)
                dt = outp.tile([128, 256], WDT, tag="dt")
                nc.scalar.activation(out=dt[:], in_=psd[:], func=SILU, bias=0.0)
                r0 = t0 + half * 512
                nc.sync.dma_start(
                    out=tbl_out[r0:r0 + 512, :].rearrange("(s p) c -> p s c", p=128),
                    in_=dt[:].rearrange("p (s c) -> p s c", c=I))
    nc.compile()
    return nc


def build_launch2(H, I, SBF, group_Ds, tbl_rows):
    """Per-core launch 2. group_Ds: list of per-group degree class
    (len % MACRO_G == 0).

    The sbf projection packs 3 chunks per matmul: sbfT is stored as
    [3*SBF, ...] vertical stacks and multiplied against a block-diagonal
    [3*SBF, 3*I] weight, so one LDW+MM yields sbf_i for 384 slots.  Two
    such matmuls share one PSUM bank so the DVE modulate runs as a single
    [128, 384] op."""
    G_total = len(group_Ds)
    assert G_total % MACRO_G == 0
    SLOT_COLS = int(sum(group_Ds))
    trip_Ds = [-(-d // 3) for d in group_Ds]          # triples per group
    TRIP_COLS = int(sum(trip_Ds))
    NSLOT3 = 128 * TRIP_COLS
    NEPAD = 128 * G_total
    Dmax = max(group_Ds)
    MTILE = 128 * MACRO_G                              # 1024

    nc = bacc.Bacc("TRN2", target_bir_lowering=False, debug=False)
    xT = _dram(nc, "xT", [H, NEPAD], WDT)
    xjiT = _dram(nc, "xjiT", [H, NEPAD], WDT)
    tbl = _dram(nc, "tbl", [tbl_rows, I], WDT)
    sbfT = _dram(nc, "sbfT", [3 * SBF, NSLOT3], WDT)
    gidx = _dram(nc, "gidx", [128, SLOT_COLS], I32)
    w_sbf = _dram(nc, "W_sbf", [3 * SBF, 3 * I], WDT)
    w_up = _dram(nc, "W_up", [I, H], WDT)
    lin_names = ["rb0_0", "rb0_1", "lin", "ra0_0", "ra0_1", "ra1_0", "ra1_1"]
    lin_w = {n: _dram(nc, f"W_{n}", [H, H], WDT) for n in lin_names}
    lin_b = {n: _dram(nc, f"b_{n}", [H]) for n in lin_names}
    hT_out = _dram(nc, "hT_out", [H, NEPAD], WDT, out=True)

    with tile.TileContext(nc) as tc, ExitStack() as ctx:
        const = ctx.enter_context(tc.tile_pool(name="const", bufs=1))
        ident = const.tile([128, 128], F32, tag="ident")
        make_identity(nc, ident[:])
        gidx_sb = const.tile([128, SLOT_COLS], I32, tag="gidx")
        nc.sync.dma_start(out=gidx_sb[:], in_=gidx[:])
        wsbf_t = _load_weight_chunks(nc, const, w_sbf, "wsbf")[0]
        wup_t = _load_weight_chunks(nc, const, w_up, "wup")
        lw = {n: _load_weight_chunks(nc, const, lin_w[n], f"w{n}") for n in lin_names}
        lb = {n: _load_bias_chunks(nc, const, lin_b[n], f"b{n}") for n in lin_names}

        sbf_pool = ctx.enter_context(tc.tile_pool(name="sbfp", bufs=3))
        g_pool = ctx.enter_context(tc.tile_pool(name="gp", bufs=3))
        m_pool = ctx.enter_context(tc.tile_pool(name="mp", bufs=2))
        agg_pool = ctx.enter_context(tc.tile_pool(name="aggp", bufs=2))
        aggT_pool = ctx.enter_context(tc.tile_pool(name="aggTp", bufs=2))
        xt_pool = ctx.enter_context(tc.tile_pool(name="xtp", bufs=2))
        h_pool = ctx.enter_context(tc.tile_pool(name="hp", bufs=2))
        # PSUM: 2*1 (sbf pairs) + 2*1 (transpose) + 2*2 (tail) = 8 banks
        ps_s = ctx.enter_context(tc.tile_pool(name="ps_s", bufs=2, space="PSUM"))
        ps_t = ctx.enter_context(tc.tile_pool(name="ps_t", bufs=2, space="PSUM"))
        ps_c = ctx.enter_context(tc.tile_pool(name="ps_c", bufs=2, space="PSUM"))

        def linear_T(rhs_tiles, w_tiles, b_tiles, out_tag, act=True):
            """outT[m,:] = silu(sum_k W[k,m]^T rhs[k,:] + b[m]) per macro-tile."""
            outs = []
            for mi, m0 in enumerate(range(0, H, 128)):
                msz = min(128, H - m0)
                ps = ps_c.tile([msz, MTILE], F32, tag="psc")
                nk = len(rhs_tiles)
                for sub in range(MTILE // 512):
                    sl = slice(sub * 512, (sub + 1) * 512)
                    for ki in range(nk):
                        rt, ksz = rhs_tiles[ki]
                        _mm(nc, ps[:, sl], w_tiles[ki][0][:, m0:m0 + msz],
                            rt[:, sl], ki == 0, ki == nk - 1)
                ot = h_pool.tile([msz, MTILE], WDT, tag=f"{out_tag}_{m0}")
                bias = b_tiles[mi][:] if b_tiles is not None else 0.0
                nc.scalar.activation(out=ot[:], in_=ps[:], func=SILU, bias=bias)
                outs.append((ot, msz))
            return outs

        def add_T(a_tiles, b_tiles, out_tag):
            outs = []
            for mi, ((at, msz), (bt, _msz2)) in enumerate(zip(a_tiles, b_tiles)):
                ot = h_pool.tile([msz, MTILE], WDT, tag=f"{out_tag}_{mi}")
                nc.vector.tensor_tensor(out=ot[:], in0=at[:], in1=bt[:], op=ADD)
                outs.append((ot, msz))
            return outs

        cb = 0      # slot-column base (gather/multiply space)
        tb = 0      # triple-column base (sbfT space)
        Tmax = -(-Dmax // 3)
        for mt in range(G_total // MACRO_G):
            aggT_sb = aggT_pool.tile([I, MTILE], WDT, tag="aggT")
            for k in range(MACRO_G):
                D = int(group_Ds[mt * MACRO_G + k])
                ntrip = -(-D // 3)
                sbf_t = sbf_pool.tile([3 * SBF, Tmax * 128], WDT, tag="sbf")
                nc.sync.dma_start(out=sbf_t[:, :ntrip * 128],
                                  in_=sbfT[:, 128 * tb:128 * (tb + ntrip)])
                g_t = g_pool.tile([128, Dmax * I], WDT, tag="g")
                nc.gpsimd.indirect_dma_start(
                    out=g_t[:, :D * I],
                    out_offset=None,
                    in_=tbl[:],
                    in_offset=bass.IndirectOffsetOnAxis(ap=gidx_sb[:, cb:cb + D], axis=0),
                )
                m_t = m_pool.tile([128, Dmax * I], WDT, tag="m")
                if CD_LAYOUT:
                    m_dc = m_t[:, :D * I].rearrange("p (c d) -> p d c", d=D)
                for pt in range(-(-ntrip // 2)):
                    t0 = 2 * pt
                    ntr = min(2, ntrip - t0)
                    s_ps = ps_s.tile([128, 384], F32, tag="s")
                    for j in range(ntr):
                        _mm(nc, s_ps[:, j * 192:(j + 1) * 192],
                            sbf_t[:, (t0 + j) * 128:(t0 + j + 1) * 128],
                            wsbf_t[0][:], True, True)
                    nd = min(6, D - 6 * pt)
                    sl = slice(6 * pt * I, (6 * pt + nd) * I)
                    if CD_LAYOUT:
                        out_ap = m_dc[:, 6 * pt:6 * pt + nd, :]
                    else:
                        out_ap = m_t[:, sl]
                    nc.vector.tensor_tensor(out=out_ap, in0=g_t[:, sl],
                                            in1=s_ps[:, :nd * I], op=MULT)
                agg_t = agg_pool.tile([128, I], F32, tag="agg")
                if CD_LAYOUT:
                    red_in = m_t[:, :D * I].rearrange("p (c d) -> p c d", c=I)
                else:
                    red_in = m_t[:, :D * I].rearrange("p (d c) -> p c d", c=I)
                nc.vector.tensor_reduce(out=agg_t[:], in_=red_in, axis=AXIS_X, op=ADD)
                aggT_ps = ps_t.tile([I, 128], F32, tag="aggT_ps")
                nc.tensor.transpose(out=aggT_ps[:], in_=agg_t[:], identity=ident[:])
                nc.vector.tensor_copy(aggT_sb[:, k * 128:(k + 1) * 128], aggT_ps[:])
                cb += D
                tb += ntrip

            # ---------------- tail MLP on this 1024-edge macro-tile ------------
            col0 = mt * MTILE
            xts, xjis = [], []
            for m0 in range(0, H, 128):
                msz = min(128, H - m0)
                xt = xt_pool.tile([msz, MTILE], WDT, tag=f"xt_{m0}")
                nc.sync.dma_start(out=xt[:], in_=xT[m0:m0 + msz, col0:col0 + MTILE])
                xts.append((xt, msz))
                xj = xt_pool.tile([msz, MTILE], WDT, tag=f"xji_{m0}")
                nc.sync.dma_start(out=xj[:], in_=xjiT[m0:m0 + msz, col0:col0 + MTILE])
                xjis.append((xj, msz))

            x_up = linear_T([(aggT_sb, I)], wup_t, None, "tb")
            h = add_T(xjis, x_up, "h")
            # res_before
            t1 = linear_T(h, lw["rb0_0"], lb["rb0_0"], "ta")
            t2 = linear_T(t1, lw["rb0_1"], lb["rb0_1"], "tb")
            h = add_T(h, t2, "h")
            # lin + skip
            s = linear_T(h, lw["lin"], lb["lin"], "ta")
            h = add_T(s, xts, "h")
            # res_after x2
            t1 = linear_T(h, lw["ra0_0"], lb["ra0_0"], "ta")
            t2 = linear_T(t1, lw["ra0_1"], lb["ra0_1"], "tb")
            h = add_T(h, t2, "h")
            t1 = linear_T(h, lw["ra1_0"], lb["ra1_0"], "ta")
            t2 = linear_T(t1, lw["ra1_1"], lb["ra1_1"], "tb")
            h = add_T(h, t2, "h")

            for (ht, msz), m0 in zip(h, range(0, H, 128)):
                nc.sync.dma_start(out=hT_out[m0:m0 + msz, col0:col0 + MTILE],
                                  in_=ht[:])
    nc.compile()
    return nc


# --------------------------------------------------------------------------
# host-side planning
# --------------------------------------------------------------------------

def _degree_ladder(maxdeg):
    base = [2, 4, 6, 8, 10, 12, 14, 16, 20, 24, 28, 32, 40, 48, 64, 96, 128]
    lad = [d for d in base if d < maxdeg]
    lad.append(int(maxdeg) if maxdeg > (lad[-1] if lad else 0) else maxdeg)
    out = sorted(set(int(d) for d in lad if d >= 1))
    return out


def _plan(idx_ji, idx_kj, n_cores, Epc, T):
    """Sort triplets by idx_ji, bucket edges by degree class, build the
    static group structure (identical across cores) and per-core layouts."""
    perm_t = np.argsort(idx_ji, kind="stable")
    ji_s = idx_ji[perm_t]
    kj_s = idx_kj[perm_t]
    bounds = np.searchsorted(ji_s, np.arange(n_cores + 1) * Epc)

    degs, starts = [], []
    for c in range(n_cores):
        lo, hi = bounds[c], bounds[c + 1]
        local = ji_s[lo:hi] - c * Epc
        deg = np.bincount(local, minlength=Epc).astype(np.int64)
        st = np.searchsorted(local, np.arange(Epc)).astype(np.int64)
        degs.append(deg)
        starts.append(st)
    maxdeg = int(max(d.max() for d in degs)) if T > 0 else 1
    ladder = _degree_ladder(max(maxdeg, 1))
    L = np.array(ladder, dtype=np.int64)

    cls, counts = [], np.zeros((n_cores, len(L)), dtype=np.int64)
    for c in range(n_cores):
        cl = np.searchsorted(L, degs[c], side="left")  # deg <= L[cl]
        cls.append(cl)
        counts[c] = np.bincount(cl, minlength=len(L))
    ng = np.ceil(counts / 128.0).astype(np.int64).max(axis=0)  # per class, max
    # pad total group count to a multiple of MACRO_G (into the smallest class)
    pad = (-int(ng.sum())) % MACRO_G
    if pad:
        nz = int(np.argmax(ng > 0)) if (ng > 0).any() else 0
        ng[nz] += pad
    group_Ds = np.repeat(L, ng)
    return {
        "perm_t": perm_t, "kj_s": kj_s, "bounds": bounds,
        "degs": degs, "starts": starts, "cls": cls,
        "ladder": L, "ng": ng, "group_Ds": group_Ds,
    }


def _build_core_arrays(plan, c, Epc, sbf_ext, E_dummy):
    """Per-core: edge slot order, gidx [128, SLOT_COLS], sbf take idx [NSLOT]."""
    L, ng = plan["ladder"], plan["ng"]
    deg, st, cl = plan["degs"][c], plan["starts"][c], plan["cls"][c]
    lo = plan["bounds"][c]
    kj_s = plan["kj_s"]
    T_zero = sbf_ext.shape[0] - 1

    edge_slots_parts, gidx_parts, take_parts = [], [], []
    for k, D in enumerate(L):
        D = int(D)
        n_slots = int(ng[k]) * 128
        if n_slots == 0:
            continue
        ids = np.where(cl == k)[0]
        e = np.full(n_slots, -1, dtype=np.int64)
        e[:len(ids)] = ids
        edge_slots_parts.append(e)
        d_ar = np.arange(D, dtype=np.int64)
        valid = (e[:, None] >= 0) & (d_ar[None, :] < np.where(e >= 0, deg[np.maximum(e, 0)], 0)[:, None])
        tri = lo + np.where(e >= 0, st[np.maximum(e, 0)], 0)[:, None] + d_ar[None, :]
        rowidx = np.where(valid, kj_s[np.where(valid, tri, 0)], E_dummy)
        take = np.where(valid, plan["perm_t"][np.where(valid, tri, 0)], T_zero)
        ngk = n_slots // 128
        gidx_parts.append(rowidx.reshape(ngk, 128, D).transpose(1, 0, 2).reshape(128, ngk * D))
        tk = take.reshape(ngk, 128, D).transpose(0, 2, 1)  # [ngk, D, 128]
        D3 = -(-D // 3) * 3  # pad chunks to whole triples for the packed matmul
        if D3 != D:
            tk = np.concatenate(
                [tk, np.full((ngk, D3 - D, 128), T_zero, np.int64)], axis=1)
        take_parts.append(tk.reshape(-1))

    edge_slots = np.concatenate(edge_slots_parts)
    gidx_c = np.ascontiguousarray(np.concatenate(gidx_parts, axis=1).astype(np.int32))
    take_c = np.concatenate(take_parts)
    return edge_slots, gidx_c, take_c


# --------------------------------------------------------------------------
# numpy reference replica (for self-tests)
# --------------------------------------------------------------------------

def _np_silu(v):
    return v * (1.0 / (1.0 + np.exp(-v)))


def np_reference(x, rbf, sbf, idx_kj, idx_ji, W_rbf1, W_rbf2, W_sbf1, W_sbf2,
                 W_kj, b_kj, W_ji, b_ji, W_down, W_up,
                 res_before_W, res_before_b, W_lin, b_lin,
                 res_after_W, res_after_b):
    x = x.astype(np.float64)
    act = _np_silu
    E = x.shape[0]
    x_ji = act(x @ W_ji + b_ji)
    x_kj = act(x @ W_kj + b_kj)
    rbf_h = (rbf @ W_rbf1) @ W_rbf2
    x_kj = x_kj * rbf_h
    x_kj = act(x_kj @ W_down)
    sbf_i = (sbf @ W_sbf1) @ W_sbf2
    m = x_kj[idx_kj] * sbf_i
    agg = np.zeros((E, m.shape[1]), np.float64)
    np.add.at(agg, idx_ji, m)
    x_kj = act(agg @ W_up)
    h = x_ji + x_kj
    for l in range(res_before_W.shape[0]):
        t = act(h @ res_before_W[l, 0] + res_before_b[l, 0])
        t = act(t @ res_before_W[l, 1] + res_before_b[l, 1])
        h = h + t
    h = act(h @ W_lin + b_lin) + x
    for l in range(res_after_W.shape[0]):
        t = act(h @ res_after_W[l, 0] + res_after_b[l, 0])
        t = act(t @ res_after_W[l, 1] + res_after_b[l, 1])
        h = h + t
    return h.astype(np.float32)


# --------------------------------------------------------------------------
# main entry
# --------------------------------------------------------------------------

def kernel(x, rbf, sbf, idx_kj, idx_ji, W_rbf1, W_rbf2, W_sbf1, W_sbf2,
           W_kj, b_kj, W_ji, b_ji, W_down, W_up,
           res_before_W, res_before_b, W_lin, b_lin,
           res_after_W, res_after_b, n_cores=N_CORES, runner=None):
    x = np.ascontiguousarray(np.asarray(x, np.float32))
    rbf = np.ascontiguousarray(np.asarray(rbf, np.float32))
    sbf = np.ascontiguousarray(np.asarray(sbf, np.float32))
    idx_kj = np.asarray(idx_kj).astype(np.int64)
    idx_ji = np.asarray(idx_ji).astype(np.int64)
    f32 = lambda a: np.ascontiguousarray(np.asarray(a, np.float32))

    E, H = x.shape
    T, SBF = sbf.shape
    NR = rbf.shape[1]
    I = np.asarray(W_down).shape[1]
    assert E % n_cores == 0, (E, n_cores)
    Epc = E // n_cores
    Epc1 = -(-Epc // 1024) * 1024  # launch-1 edge count, padded to whole tiles

    W_rbf = f32(np.asarray(W_rbf1, np.float32) @ np.asarray(W_rbf2, np.float32))
    W_sbf = f32(np.asarray(W_sbf1, np.float32) @ np.asarray(W_sbf2, np.float32))

    if runner is None:
        def runner(nc, in_maps):
            return run_bass_kernel_spmd(nc, in_maps, list(range(len(in_maps)))).results

    # ---------------- launch 1: gather table + x_ji ----------------
    wdt = _np_wdt()
    nc1 = build_launch1(Epc1, H, NR, I)
    in_maps1 = []
    for c in range(n_cores):
        sl = slice(c * Epc, (c + 1) * Epc)
        xT_p = np.zeros((H, Epc1), wdt)
        xT_p[:, :Epc] = x[sl].T
        rbfT_p = np.zeros((NR, Epc1), wdt)
        rbfT_p[:, :Epc] = rbf[sl].T
        in_maps1.append({
            "xT": xT_p, "rbfT": rbfT_p,
            "W_kj": f32(W_kj).astype(wdt), "b_kj": f32(b_kj),
            "W_ji": f32(W_ji).astype(wdt), "b_ji": f32(b_ji),
            "W_rbf": W_rbf.astype(wdt), "W_down": f32(W_down).astype(wdt),
        })
    res1 = runner(nc1, in_maps1)
    tbl = np.zeros((E + 128, I), wdt)
    xji_all = []
    for c in range(n_cores):
        tbl[c * Epc:(c + 1) * Epc] = res1[c]["tbl_out"][:Epc]
        xji_all.append(res1[c]["xji_out"])

    # ---------------- host routing / padding ----------------
    plan = _plan(idx_ji, idx_kj, n_cores, Epc, T)
    group_Ds = plan["group_Ds"]
    sbf_ext = np.concatenate([sbf.astype(wdt), np.zeros((1, SBF), wdt)], axis=0)

    in_maps2, edge_slots_all = [], []
    W_sbf3 = np.zeros((3 * SBF, 3 * I), np.float32)
    for _r in range(3):
        W_sbf3[_r * SBF:(_r + 1) * SBF, _r * I:(_r + 1) * I] = W_sbf
    wmap = {
        "W_sbf": W_sbf3.astype(wdt), "W_up": f32(W_up).astype(wdt),
        "W_rb0_0": f32(res_before_W[0, 0]).astype(wdt), "b_rb0_0": f32(res_before_b[0, 0]),
        "W_rb0_1": f32(res_before_W[0, 1]).astype(wdt), "b_rb0_1": f32(res_before_b[0, 1]),
        "W_lin": f32(W_lin).astype(wdt), "b_lin": f32(b_lin),
        "W_ra0_0": f32(res_after_W[0, 0]).astype(wdt), "b_ra0_0": f32(res_after_b[0, 0]),
        "W_ra0_1": f32(res_after_W[0, 1]).astype(wdt), "b_ra0_1": f32(res_after_b[0, 1]),
        "W_ra1_0": f32(res_after_W[1, 0]).astype(wdt), "b_ra1_0": f32(res_after_b[1, 0]),
        "W_ra1_1": f32(res_after_W[1, 1]).astype(wdt), "b_ra1_1": f32(res_after_b[1, 1]),
    }
    NEPAD = 128 * len(group_Ds)
    for c in range(n_cores):
        edge_slots, gidx_c, take_c = _build_core_arrays(plan, c, Epc, sbf_ext, E)
        assert edge_slots.shape[0] == NEPAD
        edge_slots_all.append(edge_slots)
        valid = edge_slots >= 0
        xT_pad = np.zeros((H, NEPAD), wdt)
        xT_pad[:, valid] = x[c * Epc + edge_slots[valid]].T
        xjiT_pad = np.zeros((H, NEPAD), wdt)
        xjiT_pad[:, valid] = xji_all[c][:, edge_slots[valid]]
        rows = sbf_ext[take_c]                   # [slots3, SBF]
        NT = rows.shape[0] // (3 * 128)
        sbfT_c = np.ascontiguousarray(
            rows.reshape(NT, 3, 128, SBF).transpose(1, 3, 0, 2)
            .reshape(3 * SBF, NT * 128))
        in_maps2.append({
            "xT": xT_pad, "xjiT": xjiT_pad, "tbl": tbl, "sbfT": sbfT_c,
            "gidx": gidx_c, **wmap,
        })

    nc2 = build_launch2(H, I, SBF, list(map(int, group_Ds)), E + 128)
    res2 = runner(nc2, in_maps2)

    out = np.empty((E, H), np.float32)
    for c in range(n_cores):
        hT = res2[c]["hT_out"].astype(np.float32)
        es = edge_slots_all[c]
        valid = es >= 0
        out[c * Epc + es[valid]] = hT[:, valid].T
    return out


# revision 8
# speedup vs baseline: 1.5705x; 1.5705x over previous
"""DimeNet++ interaction block on 8 Trainium2 NeuronCores (Bass/Tile).

Strategy (matches the edge/triplet data-parallel sharding hint):
  * Edges are split contiguously 8 ways (50K edges/core).
  * Launch 1 (edge-parallel): each core computes its shard of the gather
    table  x_kj_down = silu((silu(x@W_kj+b) * ((rbf@W_rbf1)@W_rbf2)) @ W_down)
    AND the pre-activated  x_ji = silu(x@W_ji + b_ji)  in transposed layout.
    The host concatenates the 8 table shards into the full [E, I] table and
    permutes x_ji into the per-core group order used by launch 2.
  * Triplets are routed to the core that owns their idx_ji edge (host sorts
    triplets by idx_ji).  Within a core, edges are bucketed by degree class
    D and packed into 128-edge groups; each edge's triplet list is padded to
    D slots (padded-CSR).  Per group the device does:
      - one indirect DMA gather of 128*D rows from the replicated table
      - ntrip small matmuls  sbf_i = sbfT_chunk.T @ (W_sbf1@W_sbf2), two
        triples packed per PSUM bank
      - DVE multiply  m = gathered * sbf_i  (one op per packed pair)
      - DVE tensor_reduce over the D axis -> agg [128 edges, I]
      - PE transpose of agg -> [I, 128] for the downstream matmuls
  * The tail MLP (W_up, residual stack) runs per 1024-edge macro-tile
    entirely in transposed-activation layout; the host undoes the edge
    permutation / transposition when assembling the full output.

Everything the device computes is fp16-in/fp32-accumulate; the only host
arithmetic is the (associativity-exact) folding of W_rbf1@W_rbf2 and
W_sbf1@W_sbf2.
"""

import math
import sys
from contextlib import ExitStack

for _p in ("/opt/trn_rl_repo",):
    if _p not in sys.path:
        sys.path.insert(0, _p)

import numpy as np

import concourse.bass as bass
import concourse.mybir as mybir
import concourse.tile as tile
from concourse import bacc
from concourse.bass_utils import run_bass_kernel_spmd
from concourse.masks import make_identity

F32 = mybir.dt.float32
I32 = mybir.dt.int32
SILU = mybir.ActivationFunctionType.Silu
MULT = mybir.AluOpType.mult
ADD = mybir.AluOpType.add
AXIS_X = mybir.AxisListType.X

N_CORES = 8
F16 = mybir.dt.float16
WDT = F16          # matmul-path dtype (single-pass PE, half DMA bytes)
MACRO_G = 8        # groups per tail macro-tile (8 * 128 = 1024 edges)
CD_LAYOUT = False  # feature-major m: faster reduce but 3x slower mul writes


def _np_wdt():
    return np.float16 if WDT == F16 else np.float32


def _mm(nc, out, lhsT, rhs, start, stop):
    nc.tensor.matmul(out=out, lhsT=lhsT, rhs=rhs, start=start, stop=stop)


# --------------------------------------------------------------------------
# device program builders
# --------------------------------------------------------------------------

def _dram(nc, name, shape, dtype=F32, out=False):
    kind = "ExternalOutput" if out else "ExternalInput"
    return nc.dram_tensor(name, list(shape), dtype, kind=kind).ap()


def _load_weight_chunks(nc, pool, dram_ap, tag, dtype=None):
    """Load a [K, M] weight into SBUF as 128-partition K-chunks."""
    K = dram_ap.shape[0]
    if dtype is None:
        dtype = WDT
    tiles = []
    for k0 in range(0, K, 128):
        ksz = min(128, K - k0)
        t = pool.tile([ksz, dram_ap.shape[1]], dtype, tag=f"{tag}_{k0}")
        nc.sync.dma_start(out=t[:], in_=dram_ap[k0:k0 + ksz, :])
        tiles.append((t, ksz))
    return tiles


def _load_bias_chunks(nc, pool, dram_ap, tag):
    """Load a [M] bias into SBUF as per-partition [msz, 1] chunks."""
    M = dram_ap.shape[0]
    tiles = []
    for m0 in range(0, M, 128):
        msz = min(128, M - m0)
        t = pool.tile([msz, 1], F32, tag=f"{tag}_{m0}")
        nc.sync.dma_start(out=t[:], in_=dram_ap[m0:m0 + msz, None])
        tiles.append(t)
    return tiles


def build_launch1(Epc, H, NR, I):
    """Per-core: xT [H, Epc], rbfT [NR, Epc] ->
    tbl_out [Epc, I] (row major) and xjiT_out [H, Epc] (transposed)."""
    TILE = 1024
    assert Epc % TILE == 0

    nc = bacc.Bacc("TRN2", target_bir_lowering=False, debug=False)
    xT = _dram(nc, "xT", [H, Epc], WDT)
    rbfT = _dram(nc, "rbfT", [NR, Epc], WDT)
    w_kj = _dram(nc, "W_kj", [H, H], WDT)
    b_kj = _dram(nc, "b_kj", [H])
    w_ji = _dram(nc, "W_ji", [H, H], WDT)
    b_ji = _dram(nc, "b_ji", [H])
    w_rbf = _dram(nc, "W_rbf", [NR, H], WDT)
    w_down = _dram(nc, "W_down", [H, I], WDT)
    tbl_out = _dram(nc, "tbl_out", [Epc, I], WDT, out=True)
    xji_out = _dram(nc, "xji_out", [H, Epc], WDT, out=True)

    with tile.TileContext(nc) as tc, ExitStack() as ctx:
        const = ctx.enter_context(tc.tile_pool(name="const", bufs=1))
        wkj_t = _load_weight_chunks(nc, const, w_kj, "wkj")
        bkj_t = _load_bias_chunks(nc, const, b_kj, "bkj")
        wji_t = _load_weight_chunks(nc, const, w_ji, "wji")
        bji_t = _load_bias_chunks(nc, const, b_ji, "bji")
        wrbf_t = _load_weight_chunks(nc, const, w_rbf, "wrbf")
        wdown_t = _load_weight_chunks(nc, const, w_down, "wdown")

        xp = ctx.enter_context(tc.tile_pool(name="xp", bufs=3))
        work = ctx.enter_context(tc.tile_pool(name="work", bufs=2))
        outp = ctx.enter_context(tc.tile_pool(name="outp", bufs=3))
        # one rotating PSUM tag (2 banks/slot * 3) + down bank * 2  = 8 banks
        ps_big = ctx.enter_context(tc.tile_pool(name="ps_big", bufs=3, space="PSUM"))
        ps_d = ctx.enter_context(tc.tile_pool(name="ps_d", bufs=2, space="PSUM"))

        for t0 in range(0, Epc, TILE):
            xts = []
            for m0 in range(0, H, 128):
                msz = min(128, H - m0)
                xt = xp.tile([msz, TILE], WDT, tag=f"x_{m0}")
                nc.sync.dma_start(out=xt[:], in_=xT[m0:m0 + msz, t0:t0 + TILE])
                xts.append((xt, msz))
            rbt = xp.tile([NR, TILE], WDT, tag="rbf")
            nc.sync.dma_start(out=rbt[:], in_=rbfT[:, t0:t0 + TILE])

            xms = []
            for mi, m0 in enumerate(range(0, H, 128)):
                msz = min(128, H - m0)
                # rbf_h chunk
                ps_r = ps_big.tile([msz, TILE], F32, tag="ps")
                for sub in range(TILE // 512):
                    sl = slice(sub * 512, (sub + 1) * 512)
                    _mm(nc, ps_r[:, sl], wrbf_t[0][0][:, m0:m0 + msz], rbt[:, sl],
                        True, True)
                rh = work.tile([msz, TILE], WDT, tag=f"rh_{m0}")
                nc.vector.tensor_copy(rh[:], ps_r[:])
                # x_kj chunk
                ps_k = ps_big.tile([msz, TILE], F32, tag="ps")
                for sub in range(TILE // 512):
                    sl = slice(sub * 512, (sub + 1) * 512)
                    for ki, (wt, ksz) in enumerate(wkj_t):
                        _mm(nc, ps_k[:, sl], wt[:, m0:m0 + msz], xts[ki][0][:, sl],
                            ki == 0, ki == len(wkj_t) - 1)
                xk = work.tile([msz, TILE], WDT, tag=f"xk_{m0}")
                nc.scalar.activation(out=xk[:], in_=ps_k[:], func=SILU,
                                     bias=bkj_t[mi][:])
                xm = work.tile([msz, TILE], WDT, tag=f"xm_{m0}")
                nc.vector.tensor_tensor(out=xm[:], in0=xk[:], in1=rh[:], op=MULT)
                xms.append((xm, msz))
                # x_ji chunk
                ps_j = ps_big.tile([msz, TILE], F32, tag="ps")
                for sub in range(TILE // 512):
                    sl = slice(sub * 512, (sub + 1) * 512)
                    for ki, (wt, ksz) in enumerate(wji_t):
                        _mm(nc, ps_j[:, sl], wt[:, m0:m0 + msz], xts[ki][0][:, sl],
                            ki == 0, ki == len(wji_t) - 1)
                xj = outp.tile([msz, TILE], WDT, tag=f"xj_{m0}")
                nc.scalar.activation(out=xj[:], in_=ps_j[:], func=SILU,
                                     bias=bji_t[mi][:])
                nc.sync.dma_start(out=xji_out[m0:m0 + msz, t0:t0 + TILE], in_=xj[:])

            # x_kj_down rows: 4 x 128-edge subtiles packed into one PSUM bank
            for half in range(TILE // 512):
                psd = ps_d.tile([128, 256], F32, tag="psd")
                for j in range(4):
                    sl = slice(half * 512 + j * 128, half * 512 + (j + 1) * 128)
                    for ki, (xm, ksz) in enumerate(xms):
                        _mm(nc, psd[:, j * I:(j + 1) * I], xm[:, sl],
                            wdown_t[ki][0][:], ki == 0, ki == len(xms) - 1)
                dt = outp.tile([128, 256], WDT, tag="dt")
                nc.scalar.activation(out=dt[:], in_=psd[:], func=SILU, bias=0.0)
                r0 = t0 + half * 512
                nc.sync.dma_start(
                    out=tbl_out[r0:r0 + 512, :].rearrange("(s p) c -> p s c", p=128),
                    in_=dt[:].rearrange("p (s c) -> p s c", c=I))
    nc.compile()
    return nc


def build_launch2(H, I, SBF, group_Ds, tbl_rows):
    """Per-core launch 2. group_Ds: list of per-group degree class
    (len % MACRO_G == 0).

    The sbf projection packs 3 chunks per matmul: sbfT is stored as
    [3*SBF, ...] vertical stacks and multiplied against a block-diagonal
    [3*SBF, 3*I] weight, so one LDW+MM yields sbf_i for 384 slots.  Two
    such matmuls share one PSUM bank so the DVE modulate runs as a single
    [128, 384] op."""
    G_total = len(group_Ds)
    assert G_total % MACRO_G == 0
    SLOT_COLS = int(sum(group_Ds))
    trip_Ds = [-(-d // 3) for d in group_Ds]          # triples per group
    TRIP_COLS = int(sum(trip_Ds))
    NSLOT3 = 128 * TRIP_COLS
    NEPAD = 128 * G_total
    Dmax = max(group_Ds)
    MTILE = 128 * MACRO_G                              # 1024

    nc = bacc.Bacc("TRN2", target_bir_lowering=False, debug=False)
    xT1 = _dram(nc, "xT1", [128, NEPAD], WDT)
    xT2p = _dram(nc, "xT2p", [128, NEPAD // 2], WDT)
    xjiT1 = _dram(nc, "xjiT1", [128, NEPAD], WDT)
    xjiT2p = _dram(nc, "xjiT2p", [128, NEPAD // 2], WDT)
    tbl = _dram(nc, "tbl", [tbl_rows, I], WDT)
    sbfT = _dram(nc, "sbfT", [3 * SBF, NSLOT3], WDT)
    gidx = _dram(nc, "gidx", [128, SLOT_COLS], I32)
    w_sbf = _dram(nc, "W_sbf", [3 * SBF, 3 * I], WDT)
    wup1 = _dram(nc, "Wup1", [I, 128], WDT)
    wup2 = _dram(nc, "Wup2", [I, 64], WDT)
    lin_names = ["rb0_0", "rb0_1", "lin", "ra0_0", "ra0_1", "ra1_0", "ra1_1"]
    # packed weight pieces: W1 [128,128], W2d [128,128] (dup'd K2 rows),
    # W3 [128,64], W4d [128,64]; b1 [128], b2p [128]
    lwd = {}
    for n in lin_names:
        lwd[n] = {
            "w1": _dram(nc, f"W1_{n}", [128, 128], WDT),
            "w2d": _dram(nc, f"W2d_{n}", [128, 128], WDT),
            "w3": _dram(nc, f"W3_{n}", [128, 64], WDT),
            "w4d": _dram(nc, f"W4d_{n}", [128, 64], WDT),
            "b1": _dram(nc, f"b1_{n}", [128]),
            "b2p": _dram(nc, f"b2p_{n}", [128]),
        }
    hT1_out = _dram(nc, "hT1_out", [128, NEPAD], WDT, out=True)
    hT2p_out = _dram(nc, "hT2p_out", [128, NEPAD // 2], WDT, out=True)

    with tile.TileContext(nc) as tc, ExitStack() as ctx:
        const = ctx.enter_context(tc.tile_pool(name="const", bufs=1))
        ident = const.tile([128, 128], F32, tag="ident")
        make_identity(nc, ident[:])
        gidx_sb = const.tile([128, SLOT_COLS], I32, tag="gidx")
        nc.sync.dma_start(out=gidx_sb[:], in_=gidx[:])
        wsbf_t = _load_weight_chunks(nc, const, w_sbf, "wsbf")[0]
        wup1_t = _load_weight_chunks(nc, const, wup1, "wup1")[0][0]
        wup2_t = _load_weight_chunks(nc, const, wup2, "wup2")[0][0]
        lw = {}
        for n in lin_names:
            lw[n] = {k: _load_weight_chunks(nc, const, lwd[n][k], f"{k}{n}")[0][0]
                     for k in ("w1", "w2d", "w3", "w4d")}
            lw[n]["b1"] = _load_bias_chunks(nc, const, lwd[n]["b1"], f"b1{n}")[0]
            lw[n]["b2p"] = _load_bias_chunks(nc, const, lwd[n]["b2p"], f"b2{n}")[0]

        sbf_pool = ctx.enter_context(tc.tile_pool(name="sbfp", bufs=3))
        g_pool = ctx.enter_context(tc.tile_pool(name="gp", bufs=3))
        m_pool = ctx.enter_context(tc.tile_pool(name="mp", bufs=2))
        agg_pool = ctx.enter_context(tc.tile_pool(name="aggp", bufs=2))
        aggT_pool = ctx.enter_context(tc.tile_pool(name="aggTp", bufs=2))
        xt_pool = ctx.enter_context(tc.tile_pool(name="xtp", bufs=2))
        h_pool = ctx.enter_context(tc.tile_pool(name="hp", bufs=2))
        # PSUM: tail 2*2 + sbf pairs 3*1 + transpose 1*1 = 8 banks
        ps_s = ctx.enter_context(tc.tile_pool(name="ps_s", bufs=3, space="PSUM"))
        ps_t = ctx.enter_context(tc.tile_pool(name="ps_t", bufs=1, space="PSUM"))
        ps_c = ctx.enter_context(tc.tile_pool(name="ps_c", bufs=2, space="PSUM"))

        HM = MTILE // 2

        def linear_P(h1, h2p, ws, out_tag):
            """Packed linear: h1 [128, MTILE] (feats 0-127), h2p [128, MTILE/2]
            (feats 128-191, col-halves stacked on partitions).  Returns the
            silu'd (o1, o2p) pair.  K64/M64 pieces ride concurrent row/col
            tiles of the PE array."""
            ps1 = ps_c.tile([128, MTILE], F32, tag="psc")
            for sub in range(2):
                sl = slice(sub * 512, (sub + 1) * 512)
                _mm(nc, ps1[:, sl], ws["w1"][:], h1[:, sl], True, False)
            _mm(nc, ps1[:, 0:512], ws["w2d"][0:64, :], h2p[0:64, :], False, True)
            _mm(nc, ps1[:, 512:1024], ws["w2d"][64:128, :], h2p[64:128, :],
                False, True)
            o1 = h_pool.tile([128, MTILE], WDT, tag=f"{out_tag}_1")
            nc.scalar.activation(out=o1[:], in_=ps1[:], func=SILU, bias=ws["b1"][:])
            ps2 = ps_c.tile([128, HM], F32, tag="psc")
            _mm(nc, ps2[0:64, :], ws["w3"][:], h1[:, 0:512], True, False)
            _mm(nc, ps2[64:128, :], ws["w3"][:], h1[:, 512:1024], True, False)
            _mm(nc, ps2[0:64, :], ws["w4d"][0:64, :], h2p[0:64, :], False, True)
            _mm(nc, ps2[64:128, :], ws["w4d"][64:128, :], h2p[64:128, :],
                False, True)
            o2 = h_pool.tile([128, HM], WDT, tag=f"{out_tag}_2")
            nc.scalar.activation(out=o2[:], in_=ps2[:], func=SILU, bias=ws["b2p"][:])
            return o1, o2

        def up_P(aggT):
            ps1 = ps_c.tile([128, MTILE], F32, tag="psc")
            for sub in range(2):
                sl = slice(sub * 512, (sub + 1) * 512)
                _mm(nc, ps1[:, sl], wup1_t[:], aggT[:, sl], True, True)
            o1 = h_pool.tile([128, MTILE], WDT, tag="tb_1")
            nc.scalar.activation(out=o1[:], in_=ps1[:], func=SILU, bias=0.0)
            ps2 = ps_c.tile([128, HM], F32, tag="psc")
            _mm(nc, ps2[0:64, :], wup2_t[:], aggT[:, 0:512], True, True)
            _mm(nc, ps2[64:128, :], wup2_t[:], aggT[:, 512:1024], True, True)
            o2 = h_pool.tile([128, HM], WDT, tag="tb_2")
            nc.scalar.activation(out=o2[:], in_=ps2[:], func=SILU, bias=0.0)
            return o1, o2

        def add_P(a, b, out_tag):
            o1 = h_pool.tile([128, MTILE], WDT, tag=f"{out_tag}_1")
            nc.vector.tensor_tensor(out=o1[:], in0=a[0][:], in1=b[0][:], op=ADD)
            o2 = h_pool.tile([128, HM], WDT, tag=f"{out_tag}_2")
            nc.vector.tensor_tensor(out=o2[:], in0=a[1][:], in1=b[1][:], op=ADD)
            return o1, o2

        cb = 0      # slot-column base (gather/multiply space)
        tb = 0      # triple-column base (sbfT space)
        Tmax = -(-Dmax // 3)
        for mt in range(G_total // MACRO_G):
            aggT_sb = aggT_pool.tile([I, MTILE], WDT, tag="aggT")
            aggT_ps = None
            for k in range(MACRO_G):
                D = int(group_Ds[mt * MACRO_G + k])
                ntrip = -(-D // 3)
                sbf_t = sbf_pool.tile([3 * SBF, Tmax * 128], WDT, tag="sbf")
                nc.sync.dma_start(out=sbf_t[:, :ntrip * 128],
                                  in_=sbfT[:, 128 * tb:128 * (tb + ntrip)])
                g_t = g_pool.tile([128, Dmax * I], WDT, tag="g")
                nc.gpsimd.indirect_dma_start(
                    out=g_t[:, :D * I],
                    out_offset=None,
                    in_=tbl[:],
                    in_offset=bass.IndirectOffsetOnAxis(ap=gidx_sb[:, cb:cb + D], axis=0),
                )
                m_t = m_pool.tile([128, Dmax * I], WDT, tag="m")
                if CD_LAYOUT:
                    m_dc = m_t[:, :D * I].rearrange("p (c d) -> p d c", d=D)
                for pt in range(-(-ntrip // 2)):
                    t0 = 2 * pt
                    ntr = min(2, ntrip - t0)
                    s_ps = ps_s.tile([128, 384], F32, tag="s")
                    for j in range(ntr):
                        _mm(nc, s_ps[:, j * 192:(j + 1) * 192],
                            sbf_t[:, (t0 + j) * 128:(t0 + j + 1) * 128],
                            wsbf_t[0][:], True, True)
                    nd = min(6, D - 6 * pt)
                    sl = slice(6 * pt * I, (6 * pt + nd) * I)
                    if CD_LAYOUT:
                        out_ap = m_dc[:, 6 * pt:6 * pt + nd, :]
                    else:
                        out_ap = m_t[:, sl]
                    nc.vector.tensor_tensor(out=out_ap, in0=g_t[:, sl],
                                            in1=s_ps[:, :nd * I], op=MULT)
                agg_t = agg_pool.tile([128, I], F32, tag="agg")
                if CD_LAYOUT:
                    red_in = m_t[:, :D * I].rearrange("p (c d) -> p c d", c=I)
                else:
                    red_in = m_t[:, :D * I].rearrange("p (d c) -> p c d", c=I)
                nc.vector.tensor_reduce(out=agg_t[:], in_=red_in, axis=AXIS_X, op=ADD)
                # two groups share one transpose PSUM bank; ACT drains it
                if k % 2 == 0:
                    aggT_ps = ps_t.tile([I, 256], F32, tag="aggT_ps")
                nc.tensor.transpose(out=aggT_ps[:, (k % 2) * 128:(k % 2 + 1) * 128],
                                    in_=agg_t[:], identity=ident[:])
                if k % 2 == 1:
                    nc.scalar.copy(out=aggT_sb[:, (k - 1) * 128:(k + 1) * 128],
                                   in_=aggT_ps[:])
                cb += D
                tb += ntrip

            # ---------------- tail MLP on this 1024-edge macro-tile ------------
            col0 = mt * MTILE
            col0h = mt * HM
            xt1 = xt_pool.tile([128, MTILE], WDT, tag="xt1")
            nc.sync.dma_start(out=xt1[:], in_=xT1[:, col0:col0 + MTILE])
            xt2 = xt_pool.tile([128, HM], WDT, tag="xt2")
            nc.sync.dma_start(out=xt2[:], in_=xT2p[:, col0h:col0h + HM])
            xj1 = xt_pool.tile([128, MTILE], WDT, tag="xj1")
            nc.sync.dma_start(out=xj1[:], in_=xjiT1[:, col0:col0 + MTILE])
            xj2 = xt_pool.tile([128, HM], WDT, tag="xj2")
            nc.sync.dma_start(out=xj2[:], in_=xjiT2p[:, col0h:col0h + HM])

            x_up = up_P(aggT_sb)
            h = add_P((xj1, xj2), x_up, "h")
            # res_before
            t1 = linear_P(h[0], h[1], lw["rb0_0"], "ta")
            t2 = linear_P(t1[0], t1[1], lw["rb0_1"], "tb")
            h = add_P(h, t2, "h")
            # lin + skip
            s = linear_P(h[0], h[1], lw["lin"], "ta")
            h = add_P(s, (xt1, xt2), "h")
            # res_after x2
            t1 = linear_P(h[0], h[1], lw["ra0_0"], "ta")
            t2 = linear_P(t1[0], t1[1], lw["ra0_1"], "tb")
            h = add_P(h, t2, "h")
            t1 = linear_P(h[0], h[1], lw["ra1_0"], "ta")
            t2 = linear_P(t1[0], t1[1], lw["ra1_1"], "tb")
            h = add_P(h, t2, "h")

            nc.sync.dma_start(out=hT1_out[:, col0:col0 + MTILE], in_=h[0][:])
            nc.sync.dma_start(out=hT2p_out[:, col0h:col0h + HM], in_=h[1][:])
    nc.compile()
    return nc


# --------------------------------------------------------------------------
# host-side planning
# --------------------------------------------------------------------------

def _degree_ladder(maxdeg):
    base = [2, 4, 6, 8, 10, 12, 14, 16, 20, 24, 28, 32, 40, 48, 64, 96, 128]
    lad = [d for d in base if d < maxdeg]
    lad.append(int(maxdeg) if maxdeg > (lad[-1] if lad else 0) else maxdeg)
    out = sorted(set(int(d) for d in lad if d >= 1))
    return out


def _plan(idx_ji, idx_kj, n_cores, Epc, T):
    """Sort triplets by idx_ji, bucket edges by degree class, build the
    static group structure (identical across cores) and per-core layouts."""
    perm_t = np.argsort(idx_ji, kind="stable")
    ji_s = idx_ji[perm_t]
    kj_s = idx_kj[perm_t]
    bounds = np.searchsorted(ji_s, np.arange(n_cores + 1) * Epc)

    degs, starts = [], []
    for c in range(n_cores):
        lo, hi = bounds[c], bounds[c + 1]
        local = ji_s[lo:hi] - c * Epc
        deg = np.bincount(local, minlength=Epc).astype(np.int64)
        st = np.searchsorted(local, np.arange(Epc)).astype(np.int64)
        degs.append(deg)
        starts.append(st)
    maxdeg = int(max(d.max() for d in degs)) if T > 0 else 1
    ladder = _degree_ladder(max(maxdeg, 1))
    L = np.array(ladder, dtype=np.int64)

    cls, counts = [], np.zeros((n_cores, len(L)), dtype=np.int64)
    for c in range(n_cores):
        cl = np.searchsorted(L, degs[c], side="left")  # deg <= L[cl]
        cls.append(cl)
        counts[c] = np.bincount(cl, minlength=len(L))
    ng = np.ceil(counts / 128.0).astype(np.int64).max(axis=0)  # per class, max
    # pad total group count to a multiple of MACRO_G (into the smallest class)
    pad = (-int(ng.sum())) % MACRO_G
    if pad:
        nz = int(np.argmax(ng > 0)) if (ng > 0).any() else 0
        ng[nz] += pad
    group_Ds = np.repeat(L, ng)
    return {
        "perm_t": perm_t, "kj_s": kj_s, "bounds": bounds,
        "degs": degs, "starts": starts, "cls": cls,
        "ladder": L, "ng": ng, "group_Ds": group_Ds,
    }


def _build_core_arrays(plan, c, Epc, sbf_ext, E_dummy):
    """Per-core: edge slot order, gidx [128, SLOT_COLS], sbf take idx [NSLOT]."""
    L, ng = plan["ladder"], plan["ng"]
    deg, st, cl = plan["degs"][c], plan["starts"][c], plan["cls"][c]
    lo = plan["bounds"][c]
    kj_s = plan["kj_s"]
    T_zero = sbf_ext.shape[0] - 1

    edge_slots_parts, gidx_parts, take_parts = [], [], []
    for k, D in enumerate(L):
        D = int(D)
        n_slots = int(ng[k]) * 128
        if n_slots == 0:
            continue
        ids = np.where(cl == k)[0]
        e = np.full(n_slots, -1, dtype=np.int64)
        e[:len(ids)] = ids
        edge_slots_parts.append(e)
        d_ar = np.arange(D, dtype=np.int64)
        valid = (e[:, None] >= 0) & (d_ar[None, :] < np.where(e >= 0, deg[np.maximum(e, 0)], 0)[:, None])
        tri = lo + np.where(e >= 0, st[np.maximum(e, 0)], 0)[:, None] + d_ar[None, :]
        rowidx = np.where(valid, kj_s[np.where(valid, tri, 0)], E_dummy)
        take = np.where(valid, plan["perm_t"][np.where(valid, tri, 0)], T_zero)
        ngk = n_slots // 128
        gidx_parts.append(rowidx.reshape(ngk, 128, D).transpose(1, 0, 2).reshape(128, ngk * D))
        tk = take.reshape(ngk, 128, D).transpose(0, 2, 1)  # [ngk, D, 128]
        D3 = -(-D // 3) * 3  # pad chunks to whole triples for the packed matmul
        if D3 != D:
            tk = np.concatenate(
                [tk, np.full((ngk, D3 - D, 128), T_zero, np.int64)], axis=1)
        take_parts.append(tk.reshape(-1))

    edge_slots = np.concatenate(edge_slots_parts)
    gidx_c = np.ascontiguousarray(np.concatenate(gidx_parts, axis=1).astype(np.int32))
    take_c = np.concatenate(take_parts)
    return edge_slots, gidx_c, take_c


# --------------------------------------------------------------------------
# numpy reference replica (for self-tests)
# --------------------------------------------------------------------------

def _np_silu(v):
    return v * (1.0 / (1.0 + np.exp(-v)))


def np_reference(x, rbf, sbf, idx_kj, idx_ji, W_rbf1, W_rbf2, W_sbf1, W_sbf2,
                 W_kj, b_kj, W_ji, b_ji, W_down, W_up,
                 res_before_W, res_before_b, W_lin, b_lin,
                 res_after_W, res_after_b):
    x = x.astype(np.float64)
    act = _np_silu
    E = x.shape[0]
    x_ji = act(x @ W_ji + b_ji)
    x_kj = act(x @ W_kj + b_kj)
    rbf_h = (rbf @ W_rbf1) @ W_rbf2
    x_kj = x_kj * rbf_h
    x_kj = act(x_kj @ W_down)
    sbf_i = (sbf @ W_sbf1) @ W_sbf2
    m = x_kj[idx_kj] * sbf_i
    agg = np.zeros((E, m.shape[1]), np.float64)
    np.add.at(agg, idx_ji, m)
    x_kj = act(agg @ W_up)
    h = x_ji + x_kj
    for l in range(res_before_W.shape[0]):
        t = act(h @ res_before_W[l, 0] + res_before_b[l, 0])
        t = act(t @ res_before_W[l, 1] + res_before_b[l, 1])
        h = h + t
    h = act(h @ W_lin + b_lin) + x
    for l in range(res_after_W.shape[0]):
        t = act(h @ res_after_W[l, 0] + res_after_b[l, 0])
        t = act(t @ res_after_W[l, 1] + res_after_b[l, 1])
        h = h + t
    return h.astype(np.float32)


# --------------------------------------------------------------------------
# main entry
# --------------------------------------------------------------------------

def kernel(x, rbf, sbf, idx_kj, idx_ji, W_rbf1, W_rbf2, W_sbf1, W_sbf2,
           W_kj, b_kj, W_ji, b_ji, W_down, W_up,
           res_before_W, res_before_b, W_lin, b_lin,
           res_after_W, res_after_b, n_cores=N_CORES, runner=None):
    x = np.ascontiguousarray(np.asarray(x, np.float32))
    rbf = np.ascontiguousarray(np.asarray(rbf, np.float32))
    sbf = np.ascontiguousarray(np.asarray(sbf, np.float32))
    idx_kj = np.asarray(idx_kj).astype(np.int64)
    idx_ji = np.asarray(idx_ji).astype(np.int64)
    f32 = lambda a: np.ascontiguousarray(np.asarray(a, np.float32))

    E, H = x.shape
    T, SBF = sbf.shape
    NR = rbf.shape[1]
    I = np.asarray(W_down).shape[1]
    assert E % n_cores == 0, (E, n_cores)
    Epc = E // n_cores
    Epc1 = -(-Epc // 1024) * 1024  # launch-1 edge count, padded to whole tiles

    W_rbf = f32(np.asarray(W_rbf1, np.float32) @ np.asarray(W_rbf2, np.float32))
    W_sbf = f32(np.asarray(W_sbf1, np.float32) @ np.asarray(W_sbf2, np.float32))

    if runner is None:
        def runner(nc, in_maps):
            return run_bass_kernel_spmd(nc, in_maps, list(range(len(in_maps)))).results

    # ---------------- launch 1: gather table + x_ji ----------------
    wdt = _np_wdt()
    nc1 = build_launch1(Epc1, H, NR, I)
    in_maps1 = []
    for c in range(n_cores):
        sl = slice(c * Epc, (c + 1) * Epc)
        xT_p = np.zeros((H, Epc1), wdt)
        xT_p[:, :Epc] = x[sl].T
        rbfT_p = np.zeros((NR, Epc1), wdt)
        rbfT_p[:, :Epc] = rbf[sl].T
        in_maps1.append({
            "xT": xT_p, "rbfT": rbfT_p,
            "W_kj": f32(W_kj).astype(wdt), "b_kj": f32(b_kj),
            "W_ji": f32(W_ji).astype(wdt), "b_ji": f32(b_ji),
            "W_rbf": W_rbf.astype(wdt), "W_down": f32(W_down).astype(wdt),
        })
    res1 = runner(nc1, in_maps1)
    tbl = np.zeros((E + 128, I), wdt)
    xji_all = []
    for c in range(n_cores):
        tbl[c * Epc:(c + 1) * Epc] = res1[c]["tbl_out"][:Epc]
        xji_all.append(res1[c]["xji_out"])

    # ---------------- host routing / padding ----------------
    plan = _plan(idx_ji, idx_kj, n_cores, Epc, T)
    group_Ds = plan["group_Ds"]
    sbf_ext = np.concatenate([sbf.astype(wdt), np.zeros((1, SBF), wdt)], axis=0)

    in_maps2, edge_slots_all = [], []
    W_sbf3 = np.zeros((3 * SBF, 3 * I), np.float32)
    for _r in range(3):
        W_sbf3[_r * SBF:(_r + 1) * SBF, _r * I:(_r + 1) * I] = W_sbf

    def _pack2(a64, mtile=1024):
        """[64, N] -> [128, N/2]: per macro-tile, the two 512-col halves are
        stacked on partitions (rows 0-63 = first half, 64-127 = second)."""
        H2, N = a64.shape
        nm = N // mtile
        return np.ascontiguousarray(
            a64.reshape(H2, nm, 2, mtile // 2).transpose(2, 0, 1, 3)
            .reshape(2 * H2, N // 2))

    def _unpack2(p, mtile=1024):
        """inverse of _pack2: [128, N/2] -> [64, N]"""
        nm = p.shape[1] // (mtile // 2)
        return p.reshape(2, 64, nm, mtile // 2).transpose(1, 2, 0, 3) \
                .reshape(64, nm * mtile)

    wmap = {"W_sbf": W_sbf3.astype(wdt)}
    W_up_ = f32(W_up)
    wmap["Wup1"] = W_up_[:, 0:128].astype(wdt)
    wmap["Wup2"] = np.ascontiguousarray(W_up_[:, 128:192]).astype(wdt)
    lin_full = {
        "rb0_0": (res_before_W[0, 0], res_before_b[0, 0]),
        "rb0_1": (res_before_W[0, 1], res_before_b[0, 1]),
        "lin": (W_lin, b_lin),
        "ra0_0": (res_after_W[0, 0], res_after_b[0, 0]),
        "ra0_1": (res_after_W[0, 1], res_after_b[0, 1]),
        "ra1_0": (res_after_W[1, 0], res_after_b[1, 0]),
        "ra1_1": (res_after_W[1, 1], res_after_b[1, 1]),
    }
    for n, (W, b) in lin_full.items():
        W = f32(W)
        b = f32(b)
        wmap[f"W1_{n}"] = np.ascontiguousarray(W[0:128, 0:128]).astype(wdt)
        wmap[f"W2d_{n}"] = np.ascontiguousarray(
            np.vstack([W[128:192, 0:128]] * 2)).astype(wdt)
        wmap[f"W3_{n}"] = np.ascontiguousarray(W[0:128, 128:192]).astype(wdt)
        wmap[f"W4d_{n}"] = np.ascontiguousarray(
            np.vstack([W[128:192, 128:192]] * 2)).astype(wdt)
        wmap[f"b1_{n}"] = np.ascontiguousarray(b[0:128])
        wmap[f"b2p_{n}"] = np.ascontiguousarray(np.concatenate([b[128:192]] * 2))

    NEPAD = 128 * len(group_Ds)
    for c in range(n_cores):
        edge_slots, gidx_c, take_c = _build_core_arrays(plan, c, Epc, sbf_ext, E)
        assert edge_slots.shape[0] == NEPAD
        edge_slots_all.append(edge_slots)
        valid = edge_slots >= 0
        xT_pad = np.zeros((H, NEPAD), wdt)
        xT_pad[:, valid] = x[c * Epc + edge_slots[valid]].T
        xjiT_pad = np.zeros((H, NEPAD), wdt)
        xjiT_pad[:, valid] = xji_all[c][:, edge_slots[valid]]
        rows = sbf_ext[take_c]                   # [slots3, SBF]
        NT = rows.shape[0] // (3 * 128)
        sbfT_c = np.ascontiguousarray(
            rows.reshape(NT, 3, 128, SBF).transpose(1, 3, 0, 2)
            .reshape(3 * SBF, NT * 128))
        in_maps2.append({
            "xT1": np.ascontiguousarray(xT_pad[0:128]),
            "xT2p": _pack2(xT_pad[128:192]),
            "xjiT1": np.ascontiguousarray(xjiT_pad[0:128]),
            "xjiT2p": _pack2(xjiT_pad[128:192]),
            "tbl": tbl, "sbfT": sbfT_c, "gidx": gidx_c, **wmap,
        })

    nc2 = build_launch2(H, I, SBF, list(map(int, group_Ds)), E + 128)
    res2 = runner(nc2, in_maps2)

    out = np.empty((E, H), np.float32)
    for c in range(n_cores):
        hT = np.concatenate(
            [res2[c]["hT1_out"], _unpack2(res2[c]["hT2p_out"])],
            axis=0).astype(np.float32)
        es = edge_slots_all[c]
        valid = es >= 0
        out[c * Epc + es[valid]] = hT[:, valid].T
    return out


# revision 10
# speedup vs baseline: 1.7730x; 1.1290x over previous
"""DimeNet++ interaction block on 8 Trainium2 NeuronCores (Bass/Tile).

Strategy (matches the edge/triplet data-parallel sharding hint):
  * Edges are split contiguously 8 ways (50K edges/core).
  * Launch 1 (edge-parallel): each core computes its shard of the gather
    table  x_kj_down = silu((silu(x@W_kj+b) * ((rbf@W_rbf1)@W_rbf2)) @ W_down)
    AND the pre-activated  x_ji = silu(x@W_ji + b_ji)  in transposed layout.
    The host concatenates the 8 table shards into the full [E, I] table and
    permutes x_ji into the per-core group order used by launch 2.
  * Triplets are routed to the core that owns their idx_ji edge (host sorts
    triplets by idx_ji).  Within a core, edges are bucketed by degree class
    D and packed into 128-edge groups; each edge's triplet list is padded to
    D slots (padded-CSR).  Per group the device does:
      - one indirect DMA gather of 128*D rows from the replicated table
      - ntrip small matmuls  sbf_i = sbfT_chunk.T @ (W_sbf1@W_sbf2), two
        triples packed per PSUM bank
      - DVE multiply  m = gathered * sbf_i  (one op per packed pair)
      - DVE tensor_reduce over the D axis -> agg [128 edges, I]
      - PE transpose of agg -> [I, 128] for the downstream matmuls
  * The tail MLP (W_up, residual stack) runs per 1024-edge macro-tile
    entirely in transposed-activation layout; the host undoes the edge
    permutation / transposition when assembling the full output.

Everything the device computes is fp16-in/fp32-accumulate; the only host
arithmetic is the (associativity-exact) folding of W_rbf1@W_rbf2 and
W_sbf1@W_sbf2.
"""

import math
import sys
from contextlib import ExitStack

for _p in ("/opt/trn_rl_repo",):
    if _p not in sys.path:
        sys.path.insert(0, _p)

import numpy as np

import concourse.bass as bass
import concourse.mybir as mybir
import concourse.tile as tile
from concourse import bacc
from concourse.bass_utils import run_bass_kernel_spmd
from concourse.masks import make_identity

F32 = mybir.dt.float32
I32 = mybir.dt.int32
SILU = mybir.ActivationFunctionType.Silu
MULT = mybir.AluOpType.mult
ADD = mybir.AluOpType.add
AXIS_X = mybir.AxisListType.X

N_CORES = 8
F16 = mybir.dt.float16
WDT = F16          # matmul-path dtype (single-pass PE, half DMA bytes)
MACRO_G = 8        # groups per tail macro-tile (8 * 128 = 1024 edges)
CD_LAYOUT = False  # feature-major m: faster reduce but 3x slower mul writes


def _np_wdt():
    return np.float16 if WDT == F16 else np.float32


def _mm(nc, out, lhsT, rhs, start, stop):
    nc.tensor.matmul(out=out, lhsT=lhsT, rhs=rhs, start=start, stop=stop)


# --------------------------------------------------------------------------
# device program builders
# --------------------------------------------------------------------------

def _dram(nc, name, shape, dtype=F32, out=False):
    kind = "ExternalOutput" if out else "ExternalInput"
    return nc.dram_tensor(name, list(shape), dtype, kind=kind).ap()


def _load_weight_chunks(nc, pool, dram_ap, tag, dtype=None):
    """Load a [K, M] weight into SBUF as 128-partition K-chunks."""
    K = dram_ap.shape[0]
    if dtype is None:
        dtype = WDT
    tiles = []
    for k0 in range(0, K, 128):
        ksz = min(128, K - k0)
        t = pool.tile([ksz, dram_ap.shape[1]], dtype, tag=f"{tag}_{k0}")
        nc.sync.dma_start(out=t[:], in_=dram_ap[k0:k0 + ksz, :])
        tiles.append((t, ksz))
    return tiles


def _load_bias_chunks(nc, pool, dram_ap, tag):
    """Load a [M] bias into SBUF as per-partition [msz, 1] chunks."""
    M = dram_ap.shape[0]
    tiles = []
    for m0 in range(0, M, 128):
        msz = min(128, M - m0)
        t = pool.tile([msz, 1], F32, tag=f"{tag}_{m0}")
        nc.sync.dma_start(out=t[:], in_=dram_ap[m0:m0 + msz, None])
        tiles.append(t)
    return tiles


def build_launch1(Epc, H, NR, I):
    """Per-core: xT1/xT2p (H-packed), rbfT [NR, Epc] ->
    tbl_out [Epc, I] (row major) + xji1/xji2p (H-packed transposed x_ji)."""
    TILE = 1024
    HM = TILE // 2
    assert Epc % TILE == 0

    nc = bacc.Bacc("TRN2", target_bir_lowering=False, debug=False)
    xT1 = _dram(nc, "xT1", [128, Epc], WDT)
    xT2p = _dram(nc, "xT2p", [128, Epc // 2], WDT)
    rbfT = _dram(nc, "rbfT", [NR, Epc], WDT)
    wk = {}
    for n in ("kj", "ji"):
        wk[n] = {
            "w1": _dram(nc, f"W1_{n}", [128, 128], WDT),
            "w2d": _dram(nc, f"W2d_{n}", [128, 128], WDT),
            "w3": _dram(nc, f"W3_{n}", [128, 64], WDT),
            "w4d": _dram(nc, f"W4d_{n}", [128, 64], WDT),
            "b1": _dram(nc, f"b1_{n}", [128]),
            "b2p": _dram(nc, f"b2p_{n}", [128]),
        }
    w_rbf = _dram(nc, "W_rbf", [NR, H], WDT)
    wdown1 = _dram(nc, "Wdown1", [128, I], WDT)
    wdown2d = _dram(nc, "Wdown2d", [128, I], WDT)
    tbl_out = _dram(nc, "tbl_out", [Epc, I], WDT, out=True)
    xji_out1 = _dram(nc, "xji_out1", [128, Epc], WDT, out=True)
    xji_out2p = _dram(nc, "xji_out2p", [128, Epc // 2], WDT, out=True)

    with tile.TileContext(nc) as tc, ExitStack() as ctx:
        const = ctx.enter_context(tc.tile_pool(name="const", bufs=1))
        lw = {}
        for n in ("kj", "ji"):
            lw[n] = {k: _load_weight_chunks(nc, const, wk[n][k], f"{k}{n}")[0][0]
                     for k in ("w1", "w2d", "w3", "w4d")}
            lw[n]["b1"] = _load_bias_chunks(nc, const, wk[n]["b1"], f"b1{n}")[0]
            lw[n]["b2p"] = _load_bias_chunks(nc, const, wk[n]["b2p"], f"b2{n}")[0]
        wrbf_t = _load_weight_chunks(nc, const, w_rbf, "wrbf")[0][0]
        wd1_t = _load_weight_chunks(nc, const, wdown1, "wd1")[0][0]
        wd2_t = _load_weight_chunks(nc, const, wdown2d, "wd2")[0][0]

        xp = ctx.enter_context(tc.tile_pool(name="xp", bufs=3))
        work = ctx.enter_context(tc.tile_pool(name="work", bufs=2))
        outp = ctx.enter_context(tc.tile_pool(name="outp", bufs=3))
        # rotating PSUM tag (2 banks/slot * 3) + down bank * 2 = 8 banks
        ps_big = ctx.enter_context(tc.tile_pool(name="ps_big", bufs=3, space="PSUM"))
        ps_d = ctx.enter_context(tc.tile_pool(name="ps_d", bufs=2, space="PSUM"))

        def linear_P1(x1, x2p, ws, out_tag, store=None):
            """Packed K192->M192 linear + silu on one 1024-edge tile."""
            ps1 = ps_big.tile([128, TILE], F32, tag="ps")
            for sub in range(2):
                sl = slice(sub * 512, (sub + 1) * 512)
                _mm(nc, ps1[:, sl], ws["w1"][:], x1[:, sl], True, False)
            _mm(nc, ps1[:, 0:512], ws["w2d"][0:64, :], x2p[0:64, :], False, True)
            _mm(nc, ps1[:, 512:1024], ws["w2d"][64:128, :], x2p[64:128, :],
                False, True)
            o1 = (outp if store else work).tile([128, TILE], WDT, tag=f"{out_tag}_1")
            nc.scalar.activation(out=o1[:], in_=ps1[:], func=SILU, bias=ws["b1"][:])
            ps2 = ps_big.tile([128, HM], F32, tag="ps")
            _mm(nc, ps2[0:64, :], ws["w3"][:], x1[:, 0:512], True, False)
            _mm(nc, ps2[64:128, :], ws["w3"][:], x1[:, 512:1024], True, False)
            _mm(nc, ps2[0:64, :], ws["w4d"][0:64, :], x2p[0:64, :], False, True)
            _mm(nc, ps2[64:128, :], ws["w4d"][64:128, :], x2p[64:128, :],
                False, True)
            o2 = (outp if store else work).tile([128, HM], WDT, tag=f"{out_tag}_2")
            nc.scalar.activation(out=o2[:], in_=ps2[:], func=SILU, bias=ws["b2p"][:])
            return o1, o2

        for ti, t0 in enumerate(range(0, Epc, TILE)):
            t0h = ti * HM
            x1 = xp.tile([128, TILE], WDT, tag="x1")
            nc.sync.dma_start(out=x1[:], in_=xT1[:, t0:t0 + TILE])
            x2p = xp.tile([128, HM], WDT, tag="x2p")
            nc.sync.dma_start(out=x2p[:], in_=xT2p[:, t0h:t0h + HM])
            rbt = xp.tile([NR, TILE], WDT, tag="rbf")
            nc.sync.dma_start(out=rbt[:], in_=rbfT[:, t0:t0 + TILE])

            # rbf_h (packed output shape)
            ps_r = ps_big.tile([128, TILE], F32, tag="ps")
            for sub in range(2):
                sl = slice(sub * 512, (sub + 1) * 512)
                _mm(nc, ps_r[:, sl], wrbf_t[:, 0:128], rbt[:, sl], True, True)
            rh1 = work.tile([128, TILE], WDT, tag="rh_1")
            nc.vector.tensor_copy(rh1[:], ps_r[:])
            ps_r2 = ps_big.tile([128, HM], F32, tag="ps")
            _mm(nc, ps_r2[0:64, :], wrbf_t[:, 128:192], rbt[:, 0:512], True, True)
            _mm(nc, ps_r2[64:128, :], wrbf_t[:, 128:192], rbt[:, 512:1024],
                True, True)
            rh2 = work.tile([128, HM], WDT, tag="rh_2")
            nc.vector.tensor_copy(rh2[:], ps_r2[:])

            # x_kj * rbf_h
            xk1, xk2 = linear_P1(x1, x2p, lw["kj"], "xk")
            xm1 = work.tile([128, TILE], WDT, tag="xm_1")
            nc.vector.tensor_tensor(out=xm1[:], in0=xk1[:], in1=rh1[:], op=MULT)
            xm2 = work.tile([128, HM], WDT, tag="xm_2")
            nc.vector.tensor_tensor(out=xm2[:], in0=xk2[:], in1=rh2[:], op=MULT)

            # x_ji
            xj1, xj2 = linear_P1(x1, x2p, lw["ji"], "xj", store=True)
            nc.sync.dma_start(out=xji_out1[:, t0:t0 + TILE], in_=xj1[:])
            nc.sync.dma_start(out=xji_out2p[:, t0h:t0h + HM], in_=xj2[:])

            # x_kj_down rows: 4 x 128-edge subtiles packed into one PSUM bank
            for half in range(2):
                psd = ps_d.tile([128, 256], F32, tag="psd")
                for j in range(4):
                    sl = slice(half * 512 + j * 128, half * 512 + (j + 1) * 128)
                    sl2 = slice(j * 128, (j + 1) * 128)
                    _mm(nc, psd[:, j * I:(j + 1) * I], xm1[:, sl],
                        wd1_t[:], True, False)
                    hb = half * 64
                    _mm(nc, psd[:, j * I:(j + 1) * I],
                        xm2[hb:hb + 64, sl2], wd2_t[hb:hb + 64, :], False, True)
                dt = outp.tile([128, 256], WDT, tag="dt")
                nc.scalar.activation(out=dt[:], in_=psd[:], func=SILU, bias=0.0)
                r0 = t0 + half * 512
                nc.sync.dma_start(
                    out=tbl_out[r0:r0 + 512, :].rearrange("(s p) c -> p s c", p=128),
                    in_=dt[:].rearrange("p (s c) -> p s c", c=I))
    nc.compile()
    return nc


def build_launch2(H, I, SBF, group_Ds, tbl_rows):
    """Per-core launch 2. group_Ds: list of per-group degree class
    (len % MACRO_G == 0).

    The sbf projection packs 3 chunks per matmul: sbfT is stored as
    [3*SBF, ...] vertical stacks and multiplied against a block-diagonal
    [3*SBF, 3*I] weight, so one LDW+MM yields sbf_i for 384 slots.  Two
    such matmuls share one PSUM bank so the DVE modulate runs as a single
    [128, 384] op."""
    G_total = len(group_Ds)
    assert G_total % MACRO_G == 0
    SLOT_COLS = int(sum(group_Ds))
    trip_Ds = [-(-d // 3) for d in group_Ds]          # triples per group
    TRIP_COLS = int(sum(trip_Ds))
    NSLOT3 = 128 * TRIP_COLS
    NEPAD = 128 * G_total
    Dmax = max(group_Ds)
    MTILE = 128 * MACRO_G                              # 1024

    nc = bacc.Bacc("TRN2", target_bir_lowering=False, debug=False)
    xT1 = _dram(nc, "xT1", [128, NEPAD], WDT)
    xT2p = _dram(nc, "xT2p", [128, NEPAD // 2], WDT)
    xjiT1 = _dram(nc, "xjiT1", [128, NEPAD], WDT)
    xjiT2p = _dram(nc, "xjiT2p", [128, NEPAD // 2], WDT)
    tbl = _dram(nc, "tbl", [tbl_rows, I], WDT)
    sbfT = _dram(nc, "sbfT", [3 * SBF, NSLOT3], WDT)
    gidx = _dram(nc, "gidx", [128, SLOT_COLS], I32)
    w_sbf = _dram(nc, "W_sbf", [3 * SBF, 3 * I], WDT)
    wup1 = _dram(nc, "Wup1", [I, 128], WDT)
    wup2 = _dram(nc, "Wup2", [I, 64], WDT)
    lin_names = ["rb0_0", "rb0_1", "lin", "ra0_0", "ra0_1", "ra1_0", "ra1_1"]
    # packed weight pieces: W1 [128,128], W2d [128,128] (dup'd K2 rows),
    # W3 [128,64], W4d [128,64]; b1 [128], b2p [128]
    lwd = {}
    for n in lin_names:
        lwd[n] = {
            "w1": _dram(nc, f"W1_{n}", [128, 128], WDT),
            "w2d": _dram(nc, f"W2d_{n}", [128, 128], WDT),
            "w3": _dram(nc, f"W3_{n}", [128, 64], WDT),
            "w4d": _dram(nc, f"W4d_{n}", [128, 64], WDT),
            "b1": _dram(nc, f"b1_{n}", [128]),
            "b2p": _dram(nc, f"b2p_{n}", [128]),
        }
    hT1_out = _dram(nc, "hT1_out", [128, NEPAD], WDT, out=True)
    hT2p_out = _dram(nc, "hT2p_out", [128, NEPAD // 2], WDT, out=True)

    with tile.TileContext(nc) as tc, ExitStack() as ctx:
        const = ctx.enter_context(tc.tile_pool(name="const", bufs=1))
        ident = const.tile([128, 128], F32, tag="ident")
        make_identity(nc, ident[:])
        gidx_sb = const.tile([128, SLOT_COLS], I32, tag="gidx")
        nc.sync.dma_start(out=gidx_sb[:], in_=gidx[:])
        wsbf_t = _load_weight_chunks(nc, const, w_sbf, "wsbf")[0]
        wup1_t = _load_weight_chunks(nc, const, wup1, "wup1")[0][0]
        wup2_t = _load_weight_chunks(nc, const, wup2, "wup2")[0][0]
        lw = {}
        for n in lin_names:
            lw[n] = {k: _load_weight_chunks(nc, const, lwd[n][k], f"{k}{n}")[0][0]
                     for k in ("w1", "w2d", "w3", "w4d")}
            lw[n]["b1"] = _load_bias_chunks(nc, const, lwd[n]["b1"], f"b1{n}")[0]
            lw[n]["b2p"] = _load_bias_chunks(nc, const, lwd[n]["b2p"], f"b2{n}")[0]

        sbf_pool = ctx.enter_context(tc.tile_pool(name="sbfp", bufs=3))
        g_pool = ctx.enter_context(tc.tile_pool(name="gp", bufs=3))
        m_pool = ctx.enter_context(tc.tile_pool(name="mp", bufs=2))
        agg_pool = ctx.enter_context(tc.tile_pool(name="aggp", bufs=2))
        aggT_pool = ctx.enter_context(tc.tile_pool(name="aggTp", bufs=2))
        xt_pool = ctx.enter_context(tc.tile_pool(name="xtp", bufs=2))
        h_pool = ctx.enter_context(tc.tile_pool(name="hp", bufs=2))
        # PSUM: tail 2*2 + sbf pairs 3*1 + transpose 1*1 = 8 banks
        ps_s = ctx.enter_context(tc.tile_pool(name="ps_s", bufs=3, space="PSUM"))
        ps_t = ctx.enter_context(tc.tile_pool(name="ps_t", bufs=1, space="PSUM"))
        ps_c = ctx.enter_context(tc.tile_pool(name="ps_c", bufs=2, space="PSUM"))

        HM = MTILE // 2

        def linear_P(h1, h2p, ws, out_tag):
            """Packed linear: h1 [128, MTILE] (feats 0-127), h2p [128, MTILE/2]
            (feats 128-191, col-halves stacked on partitions).  Returns the
            silu'd (o1, o2p) pair.  K64/M64 pieces ride concurrent row/col
            tiles of the PE array."""
            ps1 = ps_c.tile([128, MTILE], F32, tag="psc")
            for sub in range(2):
                sl = slice(sub * 512, (sub + 1) * 512)
                _mm(nc, ps1[:, sl], ws["w1"][:], h1[:, sl], True, False)
            _mm(nc, ps1[:, 0:512], ws["w2d"][0:64, :], h2p[0:64, :], False, True)
            _mm(nc, ps1[:, 512:1024], ws["w2d"][64:128, :], h2p[64:128, :],
                False, True)
            o1 = h_pool.tile([128, MTILE], WDT, tag=f"{out_tag}_1")
            nc.scalar.activation(out=o1[:], in_=ps1[:], func=SILU, bias=ws["b1"][:])
            ps2 = ps_c.tile([128, HM], F32, tag="psc")
            _mm(nc, ps2[0:64, :], ws["w3"][:], h1[:, 0:512], True, False)
            _mm(nc, ps2[64:128, :], ws["w3"][:], h1[:, 512:1024], True, False)
            _mm(nc, ps2[0:64, :], ws["w4d"][0:64, :], h2p[0:64, :], False, True)
            _mm(nc, ps2[64:128, :], ws["w4d"][64:128, :], h2p[64:128, :],
                False, True)
            o2 = h_pool.tile([128, HM], WDT, tag=f"{out_tag}_2")
            nc.scalar.activation(out=o2[:], in_=ps2[:], func=SILU, bias=ws["b2p"][:])
            return o1, o2

        def up_P(aggT):
            ps1 = ps_c.tile([128, MTILE], F32, tag="psc")
            for sub in range(2):
                sl = slice(sub * 512, (sub + 1) * 512)
                _mm(nc, ps1[:, sl], wup1_t[:], aggT[:, sl], True, True)
            o1 = h_pool.tile([128, MTILE], WDT, tag="tb_1")
            nc.scalar.activation(out=o1[:], in_=ps1[:], func=SILU, bias=0.0)
            ps2 = ps_c.tile([128, HM], F32, tag="psc")
            _mm(nc, ps2[0:64, :], wup2_t[:], aggT[:, 0:512], True, True)
            _mm(nc, ps2[64:128, :], wup2_t[:], aggT[:, 512:1024], True, True)
            o2 = h_pool.tile([128, HM], WDT, tag="tb_2")
            nc.scalar.activation(out=o2[:], in_=ps2[:], func=SILU, bias=0.0)
            return o1, o2

        def add_P(a, b, out_tag):
            o1 = h_pool.tile([128, MTILE], WDT, tag=f"{out_tag}_1")
            nc.vector.tensor_tensor(out=o1[:], in0=a[0][:], in1=b[0][:], op=ADD)
            o2 = h_pool.tile([128, HM], WDT, tag=f"{out_tag}_2")
            nc.vector.tensor_tensor(out=o2[:], in0=a[1][:], in1=b[1][:], op=ADD)
            return o1, o2

        cb = 0      # slot-column base (gather/multiply space)
        tb = 0      # triple-column base (sbfT space)
        Tmax = -(-Dmax // 3)
        for mt in range(G_total // MACRO_G):
            aggT_sb = aggT_pool.tile([I, MTILE], WDT, tag="aggT")
            aggT_ps = None
            for k in range(MACRO_G):
                D = int(group_Ds[mt * MACRO_G + k])
                ntrip = -(-D // 3)
                sbf_t = sbf_pool.tile([3 * SBF, Tmax * 128], WDT, tag="sbf")
                nc.sync.dma_start(out=sbf_t[:, :ntrip * 128],
                                  in_=sbfT[:, 128 * tb:128 * (tb + ntrip)])
                g_t = g_pool.tile([128, Dmax * I], WDT, tag="g")
                nc.gpsimd.indirect_dma_start(
                    out=g_t[:, :D * I],
                    out_offset=None,
                    in_=tbl[:],
                    in_offset=bass.IndirectOffsetOnAxis(ap=gidx_sb[:, cb:cb + D], axis=0),
                )
                m_t = m_pool.tile([128, Dmax * I], WDT, tag="m")
                if CD_LAYOUT:
                    m_dc = m_t[:, :D * I].rearrange("p (c d) -> p d c", d=D)
                for pt in range(-(-ntrip // 2)):
                    t0 = 2 * pt
                    ntr = min(2, ntrip - t0)
                    s_ps = ps_s.tile([128, 384], F32, tag="s")
                    for j in range(ntr):
                        _mm(nc, s_ps[:, j * 192:(j + 1) * 192],
                            sbf_t[:, (t0 + j) * 128:(t0 + j + 1) * 128],
                            wsbf_t[0][:], True, True)
                    nd = min(6, D - 6 * pt)
                    sl = slice(6 * pt * I, (6 * pt + nd) * I)
                    if CD_LAYOUT:
                        out_ap = m_dc[:, 6 * pt:6 * pt + nd, :]
                    else:
                        out_ap = m_t[:, sl]
                    nc.vector.tensor_tensor(out=out_ap, in0=g_t[:, sl],
                                            in1=s_ps[:, :nd * I], op=MULT)
                agg_t = agg_pool.tile([128, I], F32, tag="agg")
                if CD_LAYOUT:
                    red_in = m_t[:, :D * I].rearrange("p (c d) -> p c d", c=I)
                else:
                    red_in = m_t[:, :D * I].rearrange("p (d c) -> p c d", c=I)
                nc.vector.tensor_reduce(out=agg_t[:], in_=red_in, axis=AXIS_X, op=ADD)
                # two groups share one transpose PSUM bank; ACT drains it
                if k % 2 == 0:
                    aggT_ps = ps_t.tile([I, 256], F32, tag="aggT_ps")
                nc.tensor.transpose(out=aggT_ps[:, (k % 2) * 128:(k % 2 + 1) * 128],
                                    in_=agg_t[:], identity=ident[:])
                if k % 2 == 1:
                    nc.scalar.copy(out=aggT_sb[:, (k - 1) * 128:(k + 1) * 128],
                                   in_=aggT_ps[:])
                cb += D
                tb += ntrip

            # ---------------- tail MLP on this 1024-edge macro-tile ------------
            col0 = mt * MTILE
            col0h = mt * HM
            xt1 = xt_pool.tile([128, MTILE], WDT, tag="xt1")
            nc.sync.dma_start(out=xt1[:], in_=xT1[:, col0:col0 + MTILE])
            xt2 = xt_pool.tile([128, HM], WDT, tag="xt2")
            nc.sync.dma_start(out=xt2[:], in_=xT2p[:, col0h:col0h + HM])
            xj1 = xt_pool.tile([128, MTILE], WDT, tag="xj1")
            nc.sync.dma_start(out=xj1[:], in_=xjiT1[:, col0:col0 + MTILE])
            xj2 = xt_pool.tile([128, HM], WDT, tag="xj2")
            nc.sync.dma_start(out=xj2[:], in_=xjiT2p[:, col0h:col0h + HM])

            x_up = up_P(aggT_sb)
            h = add_P((xj1, xj2), x_up, "h")
            # res_before
            t1 = linear_P(h[0], h[1], lw["rb0_0"], "ta")
            t2 = linear_P(t1[0], t1[1], lw["rb0_1"], "tb")
            h = add_P(h, t2, "h")
            # lin + skip
            s = linear_P(h[0], h[1], lw["lin"], "ta")
            h = add_P(s, (xt1, xt2), "h")
            # res_after x2
            t1 = linear_P(h[0], h[1], lw["ra0_0"], "ta")
            t2 = linear_P(t1[0], t1[1], lw["ra0_1"], "tb")
            h = add_P(h, t2, "h")
            t1 = linear_P(h[0], h[1], lw["ra1_0"], "ta")
            t2 = linear_P(t1[0], t1[1], lw["ra1_1"], "tb")
            h = add_P(h, t2, "h")

            nc.sync.dma_start(out=hT1_out[:, col0:col0 + MTILE], in_=h[0][:])
            nc.sync.dma_start(out=hT2p_out[:, col0h:col0h + HM], in_=h[1][:])
    nc.compile()
    return nc


# --------------------------------------------------------------------------
# host-side planning
# --------------------------------------------------------------------------

def _degree_ladder(maxdeg):
    base = [2, 4, 6, 8, 10, 12, 14, 16, 20, 24, 28, 32, 40, 48, 64, 96, 128]
    lad = [d for d in base if d < maxdeg]
    lad.append(int(maxdeg) if maxdeg > (lad[-1] if lad else 0) else maxdeg)
    out = sorted(set(int(d) for d in lad if d >= 1))
    return out


def _plan(idx_ji, idx_kj, n_cores, Epc, T):
    """Sort triplets by idx_ji, bucket edges by degree class, build the
    static group structure (identical across cores) and per-core layouts."""
    perm_t = np.argsort(idx_ji, kind="stable")
    ji_s = idx_ji[perm_t]
    kj_s = idx_kj[perm_t]
    bounds = np.searchsorted(ji_s, np.arange(n_cores + 1) * Epc)

    degs, starts = [], []
    for c in range(n_cores):
        lo, hi = bounds[c], bounds[c + 1]
        local = ji_s[lo:hi] - c * Epc
        deg = np.bincount(local, minlength=Epc).astype(np.int64)
        st = np.searchsorted(local, np.arange(Epc)).astype(np.int64)
        degs.append(deg)
        starts.append(st)
    maxdeg = int(max(d.max() for d in degs)) if T > 0 else 1
    ladder = _degree_ladder(max(maxdeg, 1))
    L = np.array(ladder, dtype=np.int64)

    cls, counts = [], np.zeros((n_cores, len(L)), dtype=np.int64)
    for c in range(n_cores):
        cl = np.searchsorted(L, degs[c], side="left")  # deg <= L[cl]
        cls.append(cl)
        counts[c] = np.bincount(cl, minlength=len(L))
    ng = np.ceil(counts / 128.0).astype(np.int64).max(axis=0)  # per class, max
    # pad total group count to a multiple of MACRO_G (into the smallest class)
    pad = (-int(ng.sum())) % MACRO_G
    if pad:
        nz = int(np.argmax(ng > 0)) if (ng > 0).any() else 0
        ng[nz] += pad
    group_Ds = np.repeat(L, ng)
    return {
        "perm_t": perm_t, "kj_s": kj_s, "bounds": bounds,
        "degs": degs, "starts": starts, "cls": cls,
        "ladder": L, "ng": ng, "group_Ds": group_Ds,
    }


def _build_core_arrays(plan, c, Epc, sbf_ext, E_dummy):
    """Per-core: edge slot order, gidx [128, SLOT_COLS], sbf take idx [NSLOT]."""
    L, ng = plan["ladder"], plan["ng"]
    deg, st, cl = plan["degs"][c], plan["starts"][c], plan["cls"][c]
    lo = plan["bounds"][c]
    kj_s = plan["kj_s"]
    T_zero = sbf_ext.shape[0] - 1

    edge_slots_parts, gidx_parts, take_parts = [], [], []
    for k, D in enumerate(L):
        D = int(D)
        n_slots = int(ng[k]) * 128
        if n_slots == 0:
            continue
        ids = np.where(cl == k)[0]
        e = np.full(n_slots, -1, dtype=np.int64)
        e[:len(ids)] = ids
        edge_slots_parts.append(e)
        d_ar = np.arange(D, dtype=np.int64)
        valid = (e[:, None] >= 0) & (d_ar[None, :] < np.where(e >= 0, deg[np.maximum(e, 0)], 0)[:, None])
        tri = lo + np.where(e >= 0, st[np.maximum(e, 0)], 0)[:, None] + d_ar[None, :]
        rowidx = np.where(valid, kj_s[np.where(valid, tri, 0)], E_dummy)
        take = np.where(valid, plan["perm_t"][np.where(valid, tri, 0)], T_zero)
        ngk = n_slots // 128
        gidx_parts.append(rowidx.reshape(ngk, 128, D).transpose(1, 0, 2).reshape(128, ngk * D))
        tk = take.reshape(ngk, 128, D).transpose(0, 2, 1)  # [ngk, D, 128]
        D3 = -(-D // 3) * 3  # pad chunks to whole triples for the packed matmul
        if D3 != D:
            tk = np.concatenate(
                [tk, np.full((ngk, D3 - D, 128), T_zero, np.int64)], axis=1)
        take_parts.append(tk.reshape(-1))

    edge_slots = np.concatenate(edge_slots_parts)
    gidx_c = np.ascontiguousarray(np.concatenate(gidx_parts, axis=1).astype(np.int32))
    take_c = np.concatenate(take_parts)
    return edge_slots, gidx_c, take_c


# --------------------------------------------------------------------------
# numpy reference replica (for self-tests)
# --------------------------------------------------------------------------

def _np_silu(v):
    return v * (1.0 / (1.0 + np.exp(-v)))


def np_reference(x, rbf, sbf, idx_kj, idx_ji, W_rbf1, W_rbf2, W_sbf1, W_sbf2,
                 W_kj, b_kj, W_ji, b_ji, W_down, W_up,
                 res_before_W, res_before_b, W_lin, b_lin,
                 res_after_W, res_after_b):
    x = x.astype(np.float64)
    act = _np_silu
    E = x.shape[0]
    x_ji = act(x @ W_ji + b_ji)
    x_kj = act(x @ W_kj + b_kj)
    rbf_h = (rbf @ W_rbf1) @ W_rbf2
    x_kj = x_kj * rbf_h
    x_kj = act(x_kj @ W_down)
    sbf_i = (sbf @ W_sbf1) @ W_sbf2
    m = x_kj[idx_kj] * sbf_i
    agg = np.zeros((E, m.shape[1]), np.float64)
    np.add.at(agg, idx_ji, m)
    x_kj = act(agg @ W_up)
    h = x_ji + x_kj
    for l in range(res_before_W.shape[0]):
        t = act(h @ res_before_W[l, 0] + res_before_b[l, 0])
        t = act(t @ res_before_W[l, 1] + res_before_b[l, 1])
        h = h + t
    h = act(h @ W_lin + b_lin) + x
    for l in range(res_after_W.shape[0]):
        t = act(h @ res_after_W[l, 0] + res_after_b[l, 0])
        t = act(t @ res_after_W[l, 1] + res_after_b[l, 1])
        h = h + t
    return h.astype(np.float32)


# --------------------------------------------------------------------------
# main entry
# --------------------------------------------------------------------------

def kernel(x, rbf, sbf, idx_kj, idx_ji, W_rbf1, W_rbf2, W_sbf1, W_sbf2,
           W_kj, b_kj, W_ji, b_ji, W_down, W_up,
           res_before_W, res_before_b, W_lin, b_lin,
           res_after_W, res_after_b, n_cores=N_CORES, runner=None):
    x = np.ascontiguousarray(np.asarray(x, np.float32))
    rbf = np.ascontiguousarray(np.asarray(rbf, np.float32))
    sbf = np.ascontiguousarray(np.asarray(sbf, np.float32))
    idx_kj = np.asarray(idx_kj).astype(np.int64)
    idx_ji = np.asarray(idx_ji).astype(np.int64)
    f32 = lambda a: np.ascontiguousarray(np.asarray(a, np.float32))

    E, H = x.shape
    T, SBF = sbf.shape
    NR = rbf.shape[1]
    I = np.asarray(W_down).shape[1]
    assert E % n_cores == 0, (E, n_cores)
    Epc = E // n_cores
    Epc1 = -(-Epc // 1024) * 1024  # launch-1 edge count, padded to whole tiles

    W_rbf = f32(np.asarray(W_rbf1, np.float32) @ np.asarray(W_rbf2, np.float32))
    W_sbf = f32(np.asarray(W_sbf1, np.float32) @ np.asarray(W_sbf2, np.float32))

    if runner is None:
        def runner(nc, in_maps):
            return run_bass_kernel_spmd(nc, in_maps, list(range(len(in_maps)))).results

    # ---------------- launch 1: gather table + x_ji ----------------
    wdt = _np_wdt()

    def _pack2_(a64, mtile=1024):
        H2, N = a64.shape
        nm = N // mtile
        return np.ascontiguousarray(
            a64.reshape(H2, nm, 2, mtile // 2).transpose(2, 0, 1, 3)
            .reshape(2 * H2, N // 2))

    def _unpack2_(p, mtile=1024):
        nm = p.shape[1] // (mtile // 2)
        return p.reshape(2, 64, nm, mtile // 2).transpose(1, 2, 0, 3) \
                .reshape(64, nm * mtile)

    nc1 = build_launch1(Epc1, H, NR, I)
    w1map = {"W_rbf": W_rbf.astype(wdt)}
    W_down_ = f32(W_down)
    w1map["Wdown1"] = np.ascontiguousarray(W_down_[0:128]).astype(wdt)
    w1map["Wdown2d"] = np.ascontiguousarray(
        np.vstack([W_down_[128:192]] * 2)).astype(wdt)
    for n, (W, b) in {"kj": (W_kj, b_kj), "ji": (W_ji, b_ji)}.items():
        W = f32(W)
        b = f32(b)
        w1map[f"W1_{n}"] = np.ascontiguousarray(W[0:128, 0:128]).astype(wdt)
        w1map[f"W2d_{n}"] = np.ascontiguousarray(
            np.vstack([W[128:192, 0:128]] * 2)).astype(wdt)
        w1map[f"W3_{n}"] = np.ascontiguousarray(W[0:128, 128:192]).astype(wdt)
        w1map[f"W4d_{n}"] = np.ascontiguousarray(
            np.vstack([W[128:192, 128:192]] * 2)).astype(wdt)
        w1map[f"b1_{n}"] = np.ascontiguousarray(b[0:128])
        w1map[f"b2p_{n}"] = np.ascontiguousarray(np.concatenate([b[128:192]] * 2))
    in_maps1 = []
    for c in range(n_cores):
        sl = slice(c * Epc, (c + 1) * Epc)
        xT_p = np.zeros((H, Epc1), wdt)
        xT_p[:, :Epc] = x[sl].T
        rbfT_p = np.zeros((NR, Epc1), wdt)
        rbfT_p[:, :Epc] = rbf[sl].T
        in_maps1.append({
            "xT1": np.ascontiguousarray(xT_p[0:128]),
            "xT2p": _pack2_(xT_p[128:192]),
            "rbfT": rbfT_p, **w1map,
        })
    res1 = runner(nc1, in_maps1)
    tbl = np.zeros((E + 128, I), wdt)
    xji_all = []
    for c in range(n_cores):
        tbl[c * Epc:(c + 1) * Epc] = res1[c]["tbl_out"][:Epc]
        xji_all.append(np.concatenate(
            [res1[c]["xji_out1"], _unpack2_(res1[c]["xji_out2p"])], axis=0))

    # ---------------- host routing / padding ----------------
    plan = _plan(idx_ji, idx_kj, n_cores, Epc, T)
    group_Ds = plan["group_Ds"]
    sbf_ext = np.concatenate([sbf.astype(wdt), np.zeros((1, SBF), wdt)], axis=0)

    in_maps2, edge_slots_all = [], []
    W_sbf3 = np.zeros((3 * SBF, 3 * I), np.float32)
    for _r in range(3):
        W_sbf3[_r * SBF:(_r + 1) * SBF, _r * I:(_r + 1) * I] = W_sbf

    def _pack2(a64, mtile=1024):
        """[64, N] -> [128, N/2]: per macro-tile, the two 512-col halves are
        stacked on partitions (rows 0-63 = first half, 64-127 = second)."""
        H2, N = a64.shape
        nm = N // mtile
        return np.ascontiguousarray(
            a64.reshape(H2, nm, 2, mtile // 2).transpose(2, 0, 1, 3)
            .reshape(2 * H2, N // 2))

    def _unpack2(p, mtile=1024):
        """inverse of _pack2: [128, N/2] -> [64, N]"""
        nm = p.shape[1] // (mtile // 2)
        return p.reshape(2, 64, nm, mtile // 2).transpose(1, 2, 0, 3) \
                .reshape(64, nm * mtile)

    wmap = {"W_sbf": W_sbf3.astype(wdt)}
    W_up_ = f32(W_up)
    wmap["Wup1"] = W_up_[:, 0:128].astype(wdt)
    wmap["Wup2"] = np.ascontiguousarray(W_up_[:, 128:192]).astype(wdt)
    lin_full = {
        "rb0_0": (res_before_W[0, 0], res_before_b[0, 0]),
        "rb0_1": (res_before_W[0, 1], res_before_b[0, 1]),
        "lin": (W_lin, b_lin),
        "ra0_0": (res_after_W[0, 0], res_after_b[0, 0]),
        "ra0_1": (res_after_W[0, 1], res_after_b[0, 1]),
        "ra1_0": (res_after_W[1, 0], res_after_b[1, 0]),
        "ra1_1": (res_after_W[1, 1], res_after_b[1, 1]),
    }
    for n, (W, b) in lin_full.items():
        W = f32(W)
        b = f32(b)
        wmap[f"W1_{n}"] = np.ascontiguousarray(W[0:128, 0:128]).astype(wdt)
        wmap[f"W2d_{n}"] = np.ascontiguousarray(
            np.vstack([W[128:192, 0:128]] * 2)).astype(wdt)
        wmap[f"W3_{n}"] = np.ascontiguousarray(W[0:128, 128:192]).astype(wdt)
        wmap[f"W4d_{n}"] = np.ascontiguousarray(
            np.vstack([W[128:192, 128:192]] * 2)).astype(wdt)
        wmap[f"b1_{n}"] = np.ascontiguousarray(b[0:128])
        wmap[f"b2p_{n}"] = np.ascontiguousarray(np.concatenate([b[128:192]] * 2))

    NEPAD = 128 * len(group_Ds)
    for c in range(n_cores):
        edge_slots, gidx_c, take_c = _build_core_arrays(plan, c, Epc, sbf_ext, E)
        assert edge_slots.shape[0] == NEPAD
        edge_slots_all.append(edge_slots)
        valid = edge_slots >= 0
        xT_pad = np.zeros((H, NEPAD), wdt)
        xT_pad[:, valid] = x[c * Epc + edge_slots[valid]].T
        xjiT_pad = np.zeros((H, NEPAD), wdt)
        xjiT_pad[:, valid] = xji_all[c][:, edge_slots[valid]]
        rows = sbf_ext[take_c]                   # [slots3, SBF]
        NT = rows.shape[0] // (3 * 128)
        sbfT_c = np.ascontiguousarray(
            rows.reshape(NT, 3, 128, SBF).transpose(1, 3, 0, 2)
            .reshape(3 * SBF, NT * 128))
        in_maps2.append({
            "xT1": np.ascontiguousarray(xT_pad[0:128]),
            "xT2p": _pack2(xT_pad[128:192]),
            "xjiT1": np.ascontiguousarray(xjiT_pad[0:128]),
            "xjiT2p": _pack2(xjiT_pad[128:192]),
            "tbl": tbl, "sbfT": sbfT_c, "gidx": gidx_c, **wmap,
        })

    nc2 = build_launch2(H, I, SBF, list(map(int, group_Ds)), E + 128)
    res2 = runner(nc2, in_maps2)

    out = np.empty((E, H), np.float32)
    for c in range(n_cores):
        hT = np.concatenate(
            [res2[c]["hT1_out"], _unpack2(res2[c]["hT2p_out"])],
            axis=0).astype(np.float32)
        es = edge_slots_all[c]
        valid = es >= 0
        out[c * Epc + es[valid]] = hT[:, valid].T
    return out
